# revision 5
# baseline (speedup 1.0000x reference)
"""Trainium2 Bass kernel for nn_LossFunction_103079215159 (triplet-style loss
with online hard-negative mining).

Math (B=8192 rows, D=256 features; x[:,0]=anchors x0, x[:,1]=positives x1):
  a = l2norm(x0), p = l2norm(x1)
  dist[i,j] = || a_i - p_j + eps ||  (via gemm expansion), diag masked +inf
  top5 smallest per row -> pick rank[i]-th (RNG-derived, data-independent)
  loss = mean relu(||a_i-p_i+eps||^2 - ||a_i-p_neg+eps||^2)

Reduction used: with s[i,j] = <x0_i, x1_j/||x1_j||> (raw-anchor units),
  d2[i,j] = 2 - 2*s[i,j]/||x0_i|| (+O(1e-6) eps terms, negligible), so
  loss_i = (2/||x0_i||) * relu(s_sel[i] - s_ii[i]) where s_sel is the
  rank[i]-th LARGEST masked s row value. Positive per-row scaling cannot
  change the row's top-k ordering, so the 2/||x0|| factor applies at the end.

Distribution: 8-way data parallel over rows. Each core receives ONLY its
[1024, 512] slab of x (fp16 over the wire), normalizes + transposes its own
positives locally, then an on-device AllGather shares the K-major scaled
positives with every core (so the full [8192,256] matrix never crosses the
host->device wire 8x). The self-match diagonal is masked by accumulating
-3e4 into the PSUM diagonal via a per-core selector input (dsel), through
the PE itself. Matmuls run in fp16 (values are O(10), fp16 has 11-bit
mantissa; rel err vs the f32 reference lands ~1e-4..1e-3, gate is 2e-2).

Host side: one fp16 cast of x (~15 ms), no other prep. The jitted 8-core
executable and the constant inputs (one-hot rank selector, diag selector)
are cached per process, so repeated kernel() calls only ship the 8.4 MB
fp16 slab and fetch the [1024, 8] per-row losses once.
"""

import base64

import numpy as np

B = 8192
D = 256
NCORES = 8
M = B // NCORES  # 1024 rows per core
RB = M // 128  # 8 row blocks per core
NG = 8  # granules of 1024 cols each
GW = 1024  # granule width

NEG_BIG = -30000.0  # diag mask; must fit fp16 and dominate |s| <= ~50

# rank[i] in {0..4}: which of the 5 nearest negatives to use per row.
# Reproduces exactly (verified):
#   k1, k2 = jax.random.split(jax.random.key(1))
#   coin = jax.random.uniform(k1, (8192,)) < 0.5
#   rank = jnp.where(coin, 0, jax.random.randint(k2, (8192,), 0, 5))
_RANK_B64 = (
    "AAIEAAAAAAAAAAIAAwAAAAAAAAAAAAMAAAIAAAMABAAAAAAAAwACAAABAAQCBAADAAACAgAEAwAC"
    "AAMEAAAAAwEEAQMAAAIAAgAAAAAAAAAEAAQAAwAABAECAAIAAAAAAgADAAACAwQABAAAAgMAAgAE"
    "AwAAAgACAAECAAEAAAECAQEBAAAABAACBAAAAAAAAAEAAAAEAQAAAAIAAgADAAEAAAAAAQAAAQME"
    "AgAAAAEEAAAAAAMAAQAAAAAEAAAEAQAAAAAAAAAAAAAAAAADAQQAAAAAAgABAAAAAAADAAADAAQA"
    "AAAAAwMAAAAEAAAAAAAAAAEAAAMAAAAAAAQAAAACAgAEAQAAAAABAAADAgABAAIAAAAAAwQCAAAD"
    "AgAAAAADAgAAAQAABAAABAAAAAAAAAIAAAEABAADAAAAAAAEAAAAAQEBAAAAAAMAAAIAAAAAAAMA"
    "AwIDAAEAAQQAAAIAAAEEAAECAAIAAAEAAAADAAIAAQICAAABAgAAAQAAAAIAAAADAAEDBAAAAQEA"
    "AgAAAAAEBAAAAAEAAgECAAIEAAAABAAEAQIABAAAAAAAAAAAAAMBAQAAAAMCAgADAAIDAwQDBAAE"
    "AAAAAAAAAAEAAAEAAwMAAAAAAAAAAAABAAAAAAAAAAEAAAADAgMAAAMAAAAAAAMAAQAAAAAAAgAA"
    "BAAAAAMBAQABAAAAAAAAAAIAAwAAAgAEAwABAAAAAAAAAAAAAAIAAgABAgAEAAABAQIAAgIDAgAE"
    "AAAAAAAAAQAABAAEAAAAAAAAAQIAAgAAAAMAAQACAAAAAAADAAQAAQABBAAEAAMABAABAQADAQAA"
    "AgABAgAEAAIAAAAAAgAAAwAAAwAAAAAEAAAAAAEAAAAAAAIEAAAAAgAABAEAAgAAAAAAAAEAAAAC"
    "AAECBAADAAAAAQAAAAIAAAAAAgMAAAAAAQAAAAQAAAAAAAMEAwEAAgEAAAAAAAAABAADAQIDAAAA"
    "AAEAAwAAAgAAAAEAAgAAAAAAAgAAAAAABAAEAAACAAIAAAQAAgADAAEAAAQAAAACAAECAwIEAAAA"
    "BAQAAAQABAMAAAQAAwIAAQMAAAQAAAACAAAEAAAABAAAAAAAAAMBAAEAAAQDAAAAAAQDAAAAAAIA"
    "AAAEAwACAAQAAgACAAACAQQAAAQDAgQDAQAAAAAEAAADBAECBAAEAAEBAAAAAAEAAgAAAwAAAgAB"
    "AwAAAgAEBAAAAAIEAAAAAwACAAIBAAABAwQAAQAAAAQAAAAAAAIAAAEBAAIAAAAAAAEAAAAAAAEB"
    "AAAAAgACAAAAAAMAAwAAAAAABAMABAMAAQQBAAQCAAEDAAAAAAIAAAAEAAMDAAAEAAEAAQAAAAAA"
    "AAICBAABAQQEAAAAAAQAAQABAAEEAAACBAAAAAMAAAAABAAAAAEBAAICAAIAAAAAAAAEBAAAAAMC"
    "AAQDAAABAAQCAAEAAAAABAQEAAIBAAAAAgAEAAEAAAIEBAACAAIAAAAABAMDBAQAAAAAAAIAAgAA"
    "AAACAAABAwMDAAAAAAAAAAACAQAAAwAAAAAEAAAAAAMAAAAAAgMAAAICAAMAAAAEAAAAAAABAAAA"
    "AAABAAAAAAMAAAEEAAIDAAEBAAQAAAMCAAAAAAAEAAACAAMAAAACAwAAAwAEAAAAAAQAAwABAAAC"
    "AwAAAAEABAQBAAIAAAIAAwAEAAEAAAACAgAAAAEEAAQAAAADAAMDAAQDBAABBAACAwAAAAAEAAMA"
    "AgQABAIAAAAEAAQCAQMAAAIBAAIAAAQEAAACAAEAAAAAAAEAAAABAAEAAAAABAAAAAAABAADAAAA"
    "BAABBAABAAADAAAAAAAAAAAAAQAAAAAAAAMAAQAAAQACAAAAAAACAAMAAAMAAwIBAAAABAAAAAMA"
    "AAAAAAABAAABAQIBAAAAAgAAAAAEAAAAAAQAAAAAAwAAAAAAAgAAAAAAAAAAAAACAgAAAAABBAAA"
    "AwACAAEDAAAAAAQAAQACAAAEAAAAAgAAAAIAAAMBAAAAAAIEAwAAAAQAAAMAAAMAAAAAAAAAAAMC"
    "BAQAAAMAAAEBAQAAAAAAAAIAAAMAAAMAAAAAAAIABAAAAAABAgAAAAAEAAQCAAIAAAIDAAMBAAAA"
    "AwAAAQADAwABAAADAAAEAwAAAAAABAMAAAEAAAAAAAAAAAAAAAAAAAAAAAACAAAAAAICAgACAAMA"
    "AAACAwAAAAIAAQAAAAAEAQAAAgAEAAEAAwAEAAAAAAAAAAQAAwAAAwAAAAQEAgAAAAMEAAAAAAAB"
    "AwQAAgADAgEDAAQDAAAAAAIAAAAAAAAAAAAABAQAAAEEBAABAAAAAQQAAAAABAAAAAMCAAAAAAAD"
    "BAAAAAEEAwIAAAADAAAAAAAEAAIAAAMBAAADAAAAAAAAAgAAAAMCAAAEAgACAAADAAAAAwABBAAD"
    "AAIAAAAAAQAABAADAAAAAAQAAQABAAMAAwADAAAAAAAAAAMEAwADAwQBAAAAAAMAAAAAAAEDAAAE"
    "AQAAAAAAAgAAAQAAAAICAAIEAAABBAACAAABAgAAAQAABAIDAgAEAAMAAAAAAAEEAAMDBAADBAAA"
    "BAAAAAADAAABAwADAAAAAAMAAAQAAQIAAAAAAwICAAIAAAIAAAAAAQAAAAICAAMAAAEAAgQAAAAA"
    "AAQAAAAABAAAAAEAAAIAAAAAAAAAAAAAAAMABAAAAAADAgAAAAAABAAABAAAAwICAAIAAAACBAAD"
    "AAAAAAADAAABAAAAAQAAAAACAgAEAAAAAAAEBAAAAAAAAAIABAQBAAAAAAAEAQAAAAIAAQADAAAD"
    "BAADAAAEBAQAAAACAAAEAAAEAAAEAAIBAAAAAgECAAAAAAMCAAIEAgADAAMAAAADAAEAAQAAAAAB"
    "BAADAQAAAAAAAQADAAAEBAIAAAIAAQIDAAACAwAAAAMAAAAAAAAAAAQABAMAAAIDAAABAgEAAAAB"
    "AAEBAAIEAwAABAACAAQAAwEAAAAAAAAAAAABAQAAAAMBBAMAAwQABAMABAAAAwMDAQQEAAABAAEB"
    "BAAAAAAAAAABAAEDAQQAAAAABAICAAIEAAMAAAAAAwADAAQDAAECAQAAAAAAAAAAAAMCAgAAAAIA"
    "AAQEAAAAAAEAAAAAAgEAAQQAAAAEBAQDBAICAAADAgIAAQAAAQABAgQCAAABAwAAAwABAAQDAAAA"
    "AAAEAAAAAgABAAAABAAABAAAAAAAAwAEAAAAAAMAAwAAAAAAAAABAAAAAwMAAQMAAAAAAgABAAAA"
    "AAMAAQAAAQACBAAAAQAAAAECAgMAAAAAAAMAAAAEAgAAAwQCAAIAAAIAAAAAAAADBAAAAQAAAAAA"
    "AAEEAAAAAAAAAgQAAAADAAADAAAAAAAAAAAAAAIBAAEEBAAAAAAEAAAAAwABAAIBAwAAAAMEAAAA"
    "AgIDBAMAAAABAAEAAAMBAAMCAAAAAAADAAIBAAADAAAAAAABAQAAAAIAAAAEAAEAAAAAAAAABAAE"
    "AAAAAAMAAgEAAQMAAAAAAAACAAMBAgABAwAAAAAEBAAAAQADAAEAAAMBAAAAAQIAAwABAgECAQMA"
    "AAAAAAACAAAAAAEAAAAAAAAEAAAAAAMEAwABAAAEAAAAAAAAAAECAQEAAAAAAAAAAAACAAAAAQAE"
    "AAQAAAACAAQAAAAAAAAAAAEAAAABAAQBAwIAAAAAAAQCAAEBAAIAAgAAAAMEAAAEAAACAQEAAAAA"
    "AAAAAAQAAQQCAAQEAgMDAAQAAAMAAAADAAAEAAEAAwAEBAQDAAACAAEAAAAABAMDAAMAAAEAAAQA"
    "AgMAAwAABAABAAIDAAQAAAICAAIAAAAAAAIEAgAAAgAEAwIAAAABAAAEAQAAAwAAAAACBAECAQAA"
    "AwAAAwQAAwQDAAAAAAACAQQDAAAAAAAEAAAAAwMBAAAAAAQAAAAAAgIAAAADBAADBAAEAAQABAAA"
    "BAAAAwQBAAAAAAACAAACAAIAAAAEAAEABAAAAgAAAAAAAAAAAAEEAAAAAwAAAQIAAAMAAQACAwQE"
    "AQABAwAAAAAAAAAAAAMBAAAABAIAAAAAAAIEAAAAAgAAAwAEAwADAAACAAEDAwQEAwAAAAAAAAAD"
    "AwACAAIDBAAABAAEAAAAAAACAgACAgICAAAAAAAAAAADAAIDAAQBAAMAAgAAAgAAAAAAAAAAAQAE"
    "AwQAAQAAAAIBAgAAAAEAAAQAAAAAAAIAAAABAQAAAwABBAADAwABAAIAAAAAAQQBAgIABAAAAAQC"
    "AAACAgMCAwQDAAAAAAACAAABAAICAAAAAgIAAAAAAQIAAAAAAAABAAAAAAAAAAAAAAIBBAQEAAQA"
    "AgQBAAEAAAAAAAAEAwAAAAAABAAAAQABAAAAAgAAAAEAAAMBAgMAAQAAAQAAAAQAAAQAAAAAAAAA"
    "AAEAAgIAAAIAAAAAAAAEAgAAAAIBAAAAAAAAAAIEAAAAAgIAAAQAAAAAAwAAAgIAAAIABAMAAQAA"
    "AAAAAAADAAAAAAAAAAADAQADBAAAAwAAAAAAAAABBAACAQAAAAABAgADAAAAAAAAAgADAAMAAAID"
    "AAIAAAAEAAAABAAAAAAAAwABAQECAwAAAAEAAAAAAAQAAAAAAAEEAAMAAAAEAAAAAAIAAwECAAAA"
    "AQAAAAABAAAAAAAABAAAAAQABAECAAIBAAECAAAAAAADAAACAgAEAAQAAAAAAAMABAAAAQEABAAA"
    "BAEAAwMEAAMAAAQABAQDBAAAAAAAAwAAAgEEAAABAAAAAAAAAAIDAgAEAQABAwACAAAEAQQEAAIA"
    "AAADAAABAgMEBAAAAAAAAgACAAAABAQAAAABAAAAAAMDAwEAAAAEAAMABAAEAwIAAAQAAQAEAAAA"
    "AgAAAAAAAAEAAAAAAAAAAwEAAAEAAgACAAAAAQADAAAAAAEAAAAAAAAABAECAAAAAAIAAAQBAgIA"
    "AwAAAAIAAAMAAAAEAAIAAAIAAQACAAAAAAAAAAAAAAMCAAADAAEBAgAAAwAAAwADAwADAAQAAAAA"
    "AAIBAwAAAQAAAAEAAAABAAAAAAAEAAEAAAQAAgQDAgEEAgMCBAAAAQIAAgAAAgIAAAABAAQAAAAA"
    "AAAAAAEAAAAAAwQAAAAAAwAEAAAAAAADAAAAAAAEAAABBAAAAAAAAwQEAAAAAgQAAAAEAgAAAAAA"
    "AAEAAAECAAAABAIEAAAAAgAAAAECAgAAAAMDAgAAAAIBAAAEAAAAAAAAAAQAAAMAAAAAAwAAAQQA"
    "AAEDAQADAAMAAAAAAAAAAAEAAAIEAAICAQAAAAIAAAAAAAEBAAEAAAAAAAACAAMDAAEAAQAAAAAA"
    "AAADAAADAAAAAAEBAwMBAwEAAAIBAAQAAAAAAAADAAAAAAEAAAMAAAABAwMAAAAAAwAABAAAAAAA"
    "AwIAAAIDBAAEAAAAAwIAAgAAAAAAAAAAAAIAAAAAAwADAAMABAMAAgQAAwAAAwAAAAAEAgADAQAE"
    "AAQAAgAEAAAAAAADAAMAAAADAgACAQQAAAAEAAEABAAAAwEABAABAgAEBAABAwMEAAAAAQAEAgEE"
    "AAMBAAAAAAAAAAAEAAAAAAEAAAABAAAAAwAAAQIAAAMAAAAAAAAAAAAAAAACAAACBAACAAAAAAIA"
    "AAICAAEAAQAAAwMAAwEBAwAEAAMDAAQCAAIEAAABBAABBAEEAAECAQMEAAAAAAACAwADBAIBAwAB"
    "AAAAAwACAgMCAAMAAAAAAwMAAAQAAAQAAQAAAAAAAAMABAQAAwAAAAEAAgABAAAABAEAAAAAAAAC"
    "AQIAAAAAAAMAAwIAAQACAQMEAwQAAAAEAAMAAQAAAAADAQABAAQAAAABAQMBAAAEAQAAAAAAAAAE"
    "AAAAAAIEAAAEAAAAAAAEAwEAAAAAAAIAAgAAAwEAAAEAAgAAAAMAAAQEAwAAAAADAQABAwAAAAAB"
    "AwADBAAEAQAAAwAABAAABAAAAAAAAAABAAAAAAMCAAAAAgEAAAQDAQAAAAMDAAAEAAIABAAAAAAA"
    "AQMEAAAAAAAAAAAAAAEEBAAEAAQDAAAAAAAAAgAAAAMAAwAAAAEAAAAAAgAAAQAAAgAEAAADBAAA"
    "AwABAAAAAwADAAICAAIAAAICAgMEAgAAAAAAAQACAAQBBAAAAQEBAAAAAAIAAAAAAgACAAIAAAAA"
    "AQAABAIDAAAAAAAAAAAAAAAEAAAAAAABAQAAAAAEAAAAAwABAwAAAAIEAAAABAEAAgMCAwACAAAC"
    "AAADAAAAAwAAAAMAAwMAAgACAAAAAAEDBAQAAwIDAAAAAAQCAgADAAADAgAAAAAAAwAAAAMBAQEA"
    "AwEAAwABAAAAAAMCAAAAAAADAAAABAQDBAAABAEAAwAAAAQEAAAAAwAAAgIBBAACAAABAAQAAAAD"
    "AAQABAICAAAEAQMAAAACBAEAAAIAAAMEAAAABAADAAAAAAIAAAMAAQAAAAABAAIAAAACAwMDAAAA"
    "AgACAAIEAAAAAAEEAAEAAAMDAAQEBAEAAAAAAAAAAAEAAgAEAAQAAAAEAAMABAABAQMAAQADAAID"
    "AAAAAAMCAgEAAwQAAgIAAAAEAAEAAAAAAAAABAAAAAAAAAQAAAAEAAAABAAAAAAAAAAAAAAAAAAA"
    "AAAEAwMAAQMAAwQAAQABAwACAAMAAAAAAAADAQAEAgAAAgIBAAQBBAAAAAAAAAQAAQAEAgAEAAIC"
    "AAIEAAIAAgAAAAADAAAABAQAAAACBAEEAwIABAACAAAAAAMABAABAAAAAAMAAAQAAAABAAMAAAAA"
    "AgACAAMAAAAAAwAAAAIAAAAAAAAAAAMEAAQEAAIAAQAAAAQDBAAAAAQABAMAAQQAAQAAAAEEAAMD"
    "AQAABAADAAAAAAABAgAAAAAABAIAAAABAAAABAABAgECAwMAAAACAgEABAABAAAAAgEBAAAEBAAC"
    "AAAAAgEAAAMAAAACAAAAAgMAAAAAAAQBAAAAAAACAQMCAAABAAADAAADAwABAAIAAAADAAADAQAA"
    "AAAABAACAAAAAAIAAAAABAMDBAQAAAAAAAQBAAQAAAAAAAAAAQAAAAEEAAMABAEAAAAEAgAAAAMA"
    "AAAAAgMCAgIAAAAAAgAAAAAAAAMAAAAAAAEAAAAAAgMBAAMAAAAABAMEAAQAAAMAAwACBAAEAAAB"
    "AAAAAAACBAQABAAEAgQAAAAEAQMDAAMAAAIEAQAEBAADAQIABAEDAAAAAgQABAADAAAAAgACBAMB"
    "AAMDAAAAAAAAAAIDAAAAAAIABAADAAAAAQAAAAAAAAAEAQAAAgABAAMDBAIBAAAABAADAAMEAwQA"
    "AAQCAAEAAwMAAAQBAAACAAABAAEAAAQCBAMBAgAAAAAAAAAABAQCAwMABAAAAAAAAAAAAAAAAQME"
    "AAAAAQAABAACAAMCAwEBAAACAgAAAgEAAAADAAAEBAAAAAAAAAABAAABAwMAAAMCAwAEAwIAAAQA"
    "BAICAAEBAAIAAAACAgIBAAAAAgQCAgAAAQQAAAAAAAAAAAMEAAADAwQABAACBAQAAwQAAQEDAQAA"
    "BAAAAAAAAwAAAAACAAMAAgMEAwEAAAAAAAEDAAAAAAIBAAQAAAMAAAMABAAEAAEEAwMAAAABBAAE"
    "AAIEAwAAAAAAAAMAAgQAAAMAAAEAAQIAAAMDBAAABAAAAAMAAAAEAAAEAAMAAAAAAAAAAAMAAAAE"
    "AAABAwAAAQAAAAEEAAAAAAIAAQAEAAAAAAADAAMAAAQDAAAAAgQCAgEAAAIBAAAAAAADBAIAAAMA"
    "AAQAAQQAAAACAAAAAAMAAgAAAQMAAAAAAQADAAIAAAAAAgAABAAAAAQEBAAEAQQAAwABAAACAAAA"
    "AAAAAAAAAAADAAAEAAABAgADAAIAAgEDAAADAAAAAAADAwQAAAMBAAAAAAAAAAAAAgABAQADAQQA"
    "BAAAAwAAAAABAAAAAAIDAAAAAwAEAAAAAQAAAAAAAwAAAAIDAAAAAwADAAQAAAEAAAECAAIABAAA"
    "BAAABAACAAMAAQAAAAIAAgIAAgAAAAQAAQACAAACAAABAAEBAAIDAAIABAAAAwEAAgMAAAAAAAMA"
    "BAACBAAAAAAABAABBAAEAAAAAQQAAQAAAAAEAgAAAAAAAwADAAAAAAAAAAMAAAAAAAEAAAAABAEA"
    "AAAEAgIAAAIAAAAAAAAAAAAAAAEEAAADAAAAAAEAAwAAAAMEAgAAAAAAAAAAAAIEAAEAAQAABAAA"
    "BAEAAAQAAwAAAwABAAIDAwQEAAAAAwQAAAQABAMAAAECAgACAAIDAAAAAQIEAAQABAQDAAAAAAAA"
    "AAAAAAAAAwABAwAAAQADAwIAAAAAAQABAAAAAAEABAQBAwABAAADAgAEAAIAAAMABAEAAAEAAQAA"
    "BAMAAwQCAwMAAQMCAwQAAwAAAAEABAAAAAEAAgEAAAAAAAAAAAAAAAAAAgAEAQAAAAEAAAAEAwAA"
    "AQIABAMEAAABAAMAAgEEAAIAAAEEAAABAAABAQAAAAAAAgIAAAAAAAADAgABBAMEAgACBAACBAQA"
    "AgADAAACAgQAAwADAwAEBAQAAAEBAAAABAECAAAAAAAABAACAAAEBAAAAAADAAAEAAMAAAIBAAAA"
    "AAQAAQAABAAAAAACAAEDAwAEBAAAAAAAAAACAQAAAAAEAAIAAAADAAAAAAIAAwAAAAEEBAAAAgAD"
    "AAAAAgEAAAQAAAEAAAAAAAIEAAMAAwQABAACAAEBAAEAAAEABAAAAAICBAQAAQAAAgIEAAAAAAAA"
    "AAAAAAAABAIBAAAAAgIAAAACAQAAAAABAAAAAAQEAgAEAAABAAAAAAAAAAEAAAMCAwAEBAMDBAAA"
    "AAABAAABAAEBAAABAwAAAAABAAABAwMAAAABAAMEAAAAAgAAAAQAAAACAAMAAAAAAAAAAAQAAAQD"
    "AAAABAABAAIAAAIAAAAAAAICAwACAwABAAAAAAQAAwADAgAAAAAAAgEABAIAAAAAAAABBAAAAAIC"
    "AAQAAAQAAAEAAwMDAAAAAQAEBAAAAAEAAAEBAAAAAgAAAwIABAADAAAEAgAAAAAABAAAAAAAAAAC"
    "AAQAAgAEAwAAAAAEAAMEBAEAAQACAAAEAAAABAAAAAAAAAAEAQQAAAQEAAQAAgAAAQEAAQAAAAQE"
    "AAABAAAAAAQABAAEAQAABAACAwACBAQEAAAAAQEAAQABAAAAAAAAAAAAAQAAAQAAAAAEAAACAAAA"
    "BAACAAEAAAAAAAMAAAIAAAMEAQAAAAIBAAIBAAAABAECAAAAAAAAAAABAAMBAAAAAwQAAgAAAwAA"
    "AwAEAQQAAwAAAQQAAwQAAAABAAABAAAEAAQAAAACAAABAAAAAAAAAQIAAAABAAAAAAICAAACAAIA"
    "AAADAgMCAAABAAAAAwACAAMABAAAAAAAAAAAAAAAAAIAAAAAAAQBAAAAAAECAQMBAAAAAAACAAAD"
    "AAAAAAQCAAQBAAACAAAAAAMAAwIAAgMAAAABAwMDBAAABAAEAAAAAAEBAAQCAQAEAAQABAIAAAID"
    "AAEAAQAAAAACAAQAAAABAAADAQECAAAAAAQAAAMABAACAAAAAAQAAAAAAAAAAQEDAAABAwQDAwIA"
    "BAAAAQADAAAAAgAEAwAABAABAQAABAABAAQAAgAAAAAAAAQAAAMBAAACBAAEAAEEAAAABAAABAAA"
    "AAAABAMDAAEBAAAAAAAEAgMAAAAEAgADAAACAgAAAAMAAAQBAQAAAQAEAgAAAAMDAAAAAAABBAAA"
    "AAAAAwQBAAIAAAABAAIAAAIABAMAAAAEAwMAAAABAAAAAwECBAAABAAAAAACAAAAAAAAAAAEAQIB"
    "AAAABAMAAAQCAwEBAgAAAAQAAQAAAAABAAAAAAIAAwACAwECAQAAAgMCAwAEAAAEAQQAAAAAAwAA"
    "AAMAAAMAAAAABAAAAAAAAAMAAAMEAAAAAAAEAAAAAAAAAAQAAwECAAQAAAAAAgAAAAAAAAAAAAAA"
    "AAAEAAADAwAAAAMCAAIAAAAAAwAAAgADAAACAAADAAAAAAMBAAEBAAECAAADAAAEAQMDBAACAAAC"
    "AAABAAACAAQAAAAAAQAAAAAAAQABAwQAAAQCAAAAAwMAAQADAAMAAAMAAAIAAAAAAAAAAAEEAAAA"
    "AAMAAAMEAAACAAAAAAMAAwIAAQMAAgIAAAIAAQAAAAAABAMAAAAAAgEAAAABAQEBAAQAAgQDAAAA"
    "BAMAAAEAAAAAAgIAAwMAAAAABAIAAAADAAECAgIAAAEBAAMBAAQAAgAAAAIAAAIAAAAAAAQEAAAD"
    "AQEEAQIDAAACAAACAAIEAAECAAAAAgMCAwACAAABAwAAAwAAAAAABAAEAAQDAAAAAAABAQEBAAAE"
    "AAAAAwAAAgAAAAADAAECAQMAAAABAAACAAAAAAAAAwMAAAIAAAIAAAEBAAIEAAAEAAAAAAAAAAMA"
    "AQQAAAMEAAMAAwMAAQAAAAAAAAMEAAQCAAIDAAMDBAQAAAAEAAEAAAMCAQACAgAAAAEDAAQAAwAA"
    "AAAAAQQAAAICBAMAAAEAAAAAAAQDAAAAAQAAAQADAAADAAAAAAAAAQAABAAAAAAAAQADAgICAQIA"
    "AAIBAAEAAwAAAAAAAAADAwAAAAAABAIAAAAAAAAEAAMABAAAAAAAAAQAAwQABAAAAAAAAAAAAwED"
    "AAMAAAAAAAAABAMAAAAAAwEAAgABAAAAAQAAAAACAAAAAAAEAQABAAABAQAAAQAAAAMAAgABAAMA"
    "AAAABAAEAQAAAAMABAAAAAEAAQAAAwQDAAACAAQEAAACAAAEBAAAAAMBAAABAAACAAAAAAQAAAAB"
    "AAADAQIBAAADAAEAAQAAAgMBAAADAAIDAAQAAAAAAQEBAQAAAgMAAAACAAAEAwABAAAAAAAEAAAD"
    "AAEEAwEAAQAAAQACAAEAAAMAAQMAAgAAAAIAAAQAAAAAAAIDAAAAAAA="
)


def _rank_to_b64():
    """(debug helper) regenerate _RANK_B64 with jax on CPU."""
    import jax
    import jax.numpy as jnp

    cpu = jax.devices("cpu")[0]
    with jax.default_device(cpu):
        k1, k2 = jax.random.split(jax.random.key(1))
        coin = jax.random.uniform(k1, (B,)) < 0.5
        rank = jnp.where(coin, 0, jax.random.randint(k2, (B,), 0, 5))
    return base64.b64encode(np.asarray(rank, dtype=np.uint8).tobytes()).decode()


_RANK_CACHE = None


def _get_rank() -> np.ndarray:
    """rank[i]: which of the 5 nearest negatives the reference picks per row.

    Must reproduce the reference's jax.random draws bit-exactly. The default
    PRNG impl here is "rbg", whose output is backend-dependent, so compute on
    the CPU backend (the grading reference runs on CPU). Falls back to the
    embedded constant (generated the same way) if jax is unavailable.
    """
    global _RANK_CACHE
    if _RANK_CACHE is not None:
        return _RANK_CACHE
    try:
        import jax
        import jax.numpy as jnp

        cpu = jax.devices("cpu")[0]
        with jax.default_device(cpu):
            k1, k2 = jax.random.split(jax.random.key(1))
            coin = jax.random.uniform(k1, (B,)) < 0.5
            rank = jnp.where(coin, 0, jax.random.randint(k2, (B,), 0, 5))
            r = np.asarray(jax.device_get(rank)).astype(np.uint8)
    except Exception:
        r = np.frombuffer(base64.b64decode(_RANK_B64), dtype=np.uint8)
    assert r.shape == (B,)
    _RANK_CACHE = r
    return r


_NC_CACHE = None


def _build_nc():
    import concourse.mybir as mybir
    import concourse.tile as tile
    from concourse import bacc
    from concourse.masks import make_identity

    F32 = mybir.dt.float32
    F16 = mybir.dt.float16
    AF = mybir.ActivationFunctionType

    nc = bacc.Bacc()
    xs = nc.dram_tensor("xs", [M, 2 * D], F16, kind="ExternalInput").ap()
    oh = nc.dram_tensor("oh", [M, 8], F32, kind="ExternalInput").ap()
    dsel = nc.dram_tensor("dsel", [128, NG], F32, kind="ExternalInput").ap()
    loss = nc.dram_tensor("loss", [128, RB], F32, kind="ExternalOutput").ap()

    with tile.TileContext(nc) as tc:
        with (
            tc.tile_pool(name="const", bufs=1) as constp,
            tc.tile_pool(name="big", bufs=1) as bigp,
            tc.tile_pool(name="small", bufs=4) as smallp,
            tc.tile_pool(name="pst", bufs=2, space="PSUM") as pst,
            tc.tile_pool(name="psg", bufs=3, space="PSUM") as psg,
            tc.tile_pool(name="dram", bufs=1, space="DRAM") as dramp,
        ):
            # ---------------- constants ----------------
            identf = constp.tile([128, 128], F32)
            make_identity(nc, identf)
            ident16 = constp.tile([128, 128], F16)
            nc.scalar.copy(ident16, identf)
            dsel_sb = constp.tile([128, NG], F32)
            nc.scalar.dma_start(dsel_sb, dsel)
            oh_sb = constp.tile([128, RB * 8], F32)
            nc.scalar.dma_start(
                oh_sb.rearrange("p (r k) -> p r k", r=RB),
                oh.rearrange("(r p) k -> p r k", p=128),
            )
            # negsel[:, g*128:(g+1)*128] = I * dsel[g]  (NEG_BIG iff g == my core)
            negf = constp.tile([128, NG * 128], F32)
            negsel = constp.tile([128, NG * 128], F16)
            for g in range(NG):
                nc.gpsimd.tensor_scalar_mul(
                    negf[:, g * 128 : (g + 1) * 128], identf, dsel_sb[:, g : g + 1]
                )
            nc.scalar.copy(negsel, negf)

            # ---------------- slab load ----------------
            # xs_sb[:, r*512 + 0:256]   = anchor rows block r (raw fp16)
            # xs_sb[:, r*512 + 256:512] = positive rows block r (raw fp16)
            xs_sb = bigp.tile([128, RB * 2 * D], F16)
            nc.sync.dma_start(
                xs_sb.rearrange("p (r d) -> p r d", r=RB),
                xs.rearrange("(r p) d -> p r d", p=128),
            )

            # ---------------- local positives: norm + scale ----------------
            np2 = constp.tile([128, RB], F32)
            nps = constp.tile([128, RB], F32)
            invnp = constp.tile([128, RB], F32)
            sq = smallp.tile([128, D], F32, tag="sq")
            for r in range(RB):
                nc.scalar.activation(
                    sq,
                    xs_sb[:, r * 512 + 256 : r * 512 + 512],
                    AF.Square,
                    accum_out=np2[:, r : r + 1],
                )
            nc.scalar.activation(nps, np2, AF.Sqrt)
            nc.vector.reciprocal(invnp, nps)
            ps16 = bigp.tile([128, RB * D], F16)  # unit positives, fp16
            for r in range(RB):
                nc.gpsimd.tensor_scalar_mul(
                    ps16[:, r * D : (r + 1) * D],
                    xs_sb[:, r * 512 + 256 : r * 512 + 512],
                    invnp[:, r : r + 1],
                )

            # ---------------- local transposes (K-major operands) -----------
            # psl: [k-chunk, col] layout of local scaled positives ([128, 2*M])
            psl = bigp.tile([128, 2 * M], F16)
            for k in range(2):
                for r4 in range(2):
                    pt = pst.tile([128, 512], F16)
                    for j in range(4):
                        r = r4 * 4 + j
                        nc.tensor.transpose(
                            pt[:, j * 128 : (j + 1) * 128],
                            ps16[:, r * D + k * 128 : r * D + k * 128 + 128],
                            ident16,
                        )
                    nc.scalar.copy(
                        psl[:, k * M + r4 * 512 : k * M + (r4 + 1) * 512], pt
                    )
            aT = [bigp.tile([128, M], F16, name=f"aT{k}") for k in range(2)]
            for k in range(2):
                for r4 in range(2):
                    pt = pst.tile([128, 512], F16)
                    for j in range(4):
                        r = r4 * 4 + j
                        nc.tensor.transpose(
                            pt[:, j * 128 : (j + 1) * 128],
                            xs_sb[:, r * 512 + k * 128 : r * 512 + k * 128 + 128],
                            ident16,
                        )
                    nc.scalar.copy(aT[k][:, r4 * 512 : (r4 + 1) * 512], pt)

            # ---------------- s_ii and anchor norms (pre-CC, off-path) ------
            # s_ii = <a_raw16, p_unit16>: same operand bits as the matmul path.
            sii = constp.tile([128, RB], F32)
            for r in range(RB):
                dot = smallp.tile([128, D], F32, tag="dot")
                nc.gpsimd.tensor_mul(
                    dot,
                    xs_sb[:, r * 512 : r * 512 + 256],
                    ps16[:, r * D : (r + 1) * D],
                )
                nc.vector.reduce_sum(sii[:, r : r + 1], dot, axis=mybir.AxisListType.X)
            na2 = constp.tile([128, RB], F32)
            na_half = constp.tile([128, RB], F32)
            inv2na = constp.tile([128, RB], F32)
            sqa = smallp.tile([128, D], F32, tag="sqa")
            for r in range(RB):
                nc.scalar.activation(
                    sqa,
                    xs_sb[:, r * 512 : r * 512 + 256],
                    AF.Square,
                    accum_out=na2[:, r : r + 1],
                )
            nc.scalar.activation(na_half, na2, AF.Sqrt, scale=0.25)
            nc.vector.reciprocal(inv2na, na_half)

            # ---------------- AllGather the K-major positives ----------------
            pslab_d = dramp.tile([2, 128, M], F16)
            nc.sync.dma_start(
                pslab_d.rearrange("k p m -> p k m"),
                psl.rearrange("p (k m) -> p k m", k=2),
            )
            pall_d = nc.dram_tensor(
                "pall_d", [NG, 2, 128, M], F16, addr_space="Shared"
            ).ap()
            nc.gpsimd.collective_compute(
                "AllGather",
                mybir.AluOpType.bypass,
                replica_groups=[list(range(NCORES))],
                ins=[pslab_d.opt()],
                outs=[pall_d.opt()],
            )
            pT = [bigp.tile([128, B], F16, name=f"pT{k}") for k in range(2)]
            for g in range(NG):
                for k in range(2):
                    nc.sync.dma_start(pT[k][:, g * M : (g + 1) * M], pall_d[g, k])

            # ---------------- main loop: matmul granules + top-8 ------------
            cand = [
                constp.tile([128, NG * 8], F32, name=f"cand{r}") for r in range(RB)
            ]
            for g in range(NG):
                for r in range(RB):
                    gt = psg.tile([128, GW], F32)
                    hd = r // 4  # 512-col half holding this row block's diagonal
                    for h in range(2):
                        for k in range(2):
                            nc.tensor.matmul(
                                gt[:, h * 512 : (h + 1) * 512],
                                aT[k][:, r * 128 : (r + 1) * 128],
                                pT[k][:, g * GW + h * 512 : g * GW + (h + 1) * 512],
                                start=(k == 0),
                                stop=(k == 1 and h != hd),
                            )
                        if h == hd:
                            # accumulate dsel[g] * I at the self-match block:
                            # NEG_BIG on the diagonal iff granule g is mine.
                            nc.tensor.matmul(
                                gt[:, r * 128 : (r + 1) * 128],
                                negsel[:, g * 128 : (g + 1) * 128],
                                ident16,
                                start=False,
                                stop=True,
                            )
                    nc.vector.max(out=cand[r][:, g * 8 : (g + 1) * 8], in_=gt)

            # ---------------- epilogue: merge, select, loss -----------------
            top8a = constp.tile([128, RB * 8], F32)
            for r in range(RB):
                nc.vector.max(out=top8a[:, r * 8 : (r + 1) * 8], in_=cand[r])
            sel_all = constp.tile([128, RB * 8], F32)
            nc.vector.tensor_mul(sel_all, top8a, oh_sb)
            selv = constp.tile([128, RB], F32)
            nc.vector.reduce_sum(
                selv,
                sel_all.rearrange("p (r k) -> p r k", r=RB),
                axis=mybir.AxisListType.X,
            )
            loss_sb = constp.tile([128, RB], F32)
            nc.vector.tensor_sub(loss_sb, selv, sii)
            # fold the anchor scale in before relu: relu(c*x) = c*relu(x)
            nc.vector.tensor_mul(loss_sb, loss_sb, inv2na)
            relu_sb = constp.tile([128, RB], F32)
            nc.scalar.activation(relu_sb, loss_sb, AF.Relu)
            nc.sync.dma_start(loss, relu_sb)

    nc.compile()
    return nc


def _get_nc():
    global _NC_CACHE
    if _NC_CACHE is None:
        _NC_CACHE = _build_nc()
    return _NC_CACHE


def _host_inputs():
    """Constant (input-independent) host arrays: one-hot rank + diag selector."""
    rank = _get_rank()
    onehot = np.zeros((B, 8), dtype=np.float32)
    onehot[np.arange(B), rank] = 1.0
    dsel = np.zeros((NCORES * 128, NG), dtype=np.float32)
    for c in range(NCORES):
        dsel[c * 128 : (c + 1) * 128, c] = NEG_BIG
    return onehot, dsel


_RT_CACHE = None


def _get_rt():
    """Cached 8-core jitted executable + device-resident constant inputs.

    Mirrors bass2jax.run_bass_via_pjrt's multi-core path, but builds the
    jitted shard_map once per process (run_bass_via_pjrt re-creates the
    closure -> re-jits + re-NEFF-compiles on every call) and keeps the
    constant operands on device.
    """
    global _RT_CACHE
    if _RT_CACHE is not None:
        return _RT_CACHE

    import jax
    from jax.experimental.shard_map import shard_map
    from jax.sharding import Mesh, NamedSharding, PartitionSpec

    import concourse.mybir as mybir
    from concourse import bass2jax

    bass2jax.install_neuronx_cc_hook()
    nc = _get_nc()
    assert nc.dbg_addr is None
    partition_name = nc.partition_id_tensor.name if nc.partition_id_tensor else None

    in_names, out_names, out_avals, zero_outs = [], [], [], []
    for alloc in nc.m.functions[0].allocations:
        if not isinstance(alloc, mybir.MemoryLocationSet):
            continue
        name = alloc.memorylocations[0].name
        if alloc.kind == "ExternalInput":
            if name != partition_name:
                in_names.append(name)
        elif alloc.kind == "ExternalOutput":
            shape = tuple(alloc.tensor_shape)
            dtype = mybir.dt.np(alloc.dtype)
            out_names.append(name)
            out_avals.append(jax.core.ShapedArray(shape, dtype))
            zero_outs.append(np.zeros(shape, dtype))
    assert in_names == ["xs", "oh", "dsel"] and out_names == ["loss"], (
        in_names,
        out_names,
    )
    n_params = len(in_names)
    all_names = list(in_names) + list(out_names)
    if partition_name is not None:
        all_names.append(partition_name)
    all_names = tuple(all_names)

    def _body(*args):
        operands = list(args)
        if partition_name is not None:
            operands.append(bass2jax.partition_id_tensor())
        outs = bass2jax._bass_exec_p.bind(
            *operands,
            out_avals=tuple(out_avals),
            in_names=all_names,
            out_names=tuple(out_names),
            lowering_input_output_aliases=(),
            sim_require_finite=True,
            sim_require_nnan=True,
            nc=nc,
        )
        return tuple(outs)

    devices = jax.devices()[:NCORES]
    assert len(devices) == NCORES, f"need {NCORES} cores, got {len(devices)}"
    mesh = Mesh(np.asarray(devices), ("core",))
    spec = PartitionSpec("core")
    n_outs = len(out_names)
    donate = tuple(range(n_params, n_params + n_outs))
    sharded = jax.jit(
        shard_map(
            _body,
            mesh=mesh,
            in_specs=(spec,) * (n_params + n_outs),
            out_specs=(spec,) * n_outs,
            check_rep=False,
        ),
        donate_argnums=donate,
        keep_unused=True,
    )

    onehot, dsel = _host_inputs()
    sh = NamedSharding(mesh, spec)
    oh_dev = jax.device_put(onehot, sh)
    dsel_dev = jax.device_put(dsel, sh)
    zero_shape = (NCORES * zero_outs[0].shape[0], *zero_outs[0].shape[1:])
    zero_dtype = zero_outs[0].dtype

    _RT_CACHE = (sharded, oh_dev, dsel_dev, zero_shape, zero_dtype)
    return _RT_CACHE


def _run_fast(x16: np.ndarray) -> np.ndarray:
    sharded, oh_dev, dsel_dev, zero_shape, zero_dtype = _get_rt()
    out = sharded(x16, oh_dev, dsel_dev, np.zeros(zero_shape, zero_dtype))
    return np.asarray(out[0])  # [NCORES*128, RB] per-row losses


def _run_spmd(x16: np.ndarray) -> np.ndarray:
    """Fallback: the stock run_bass_kernel_spmd path (re-jits per call)."""
    from concourse.bass_utils import run_bass_kernel_spmd

    onehot, dsel = _host_inputs()
    in_maps = []
    for c in range(NCORES):
        in_maps.append(
            {
                "xs": np.ascontiguousarray(x16[c * M : (c + 1) * M]),
                "oh": np.ascontiguousarray(onehot[c * M : (c + 1) * M]),
                "dsel": np.ascontiguousarray(dsel[c * 128 : (c + 1) * 128]),
            }
        )
    res = run_bass_kernel_spmd(_get_nc(), in_maps, list(range(NCORES)))
    return np.concatenate([res.results[c]["loss"] for c in range(NCORES)], axis=0)


def kernel(x: np.ndarray, _want_timing: bool = False):
    """x: [8192, 2, 256] float32 -> scalar float32 loss (0-d ndarray)."""
    x = np.asarray(x)
    assert x.shape == (B, 2, D)
    x16 = np.ascontiguousarray(x.reshape(B, 2 * D)).astype(np.float16)

    try:
        per_row = _run_fast(x16)
    except Exception as e:  # pragma: no cover - belt and braces
        import sys

        print(f"kernel: fast path failed ({type(e).__name__}: {e}); "
              f"falling back to run_bass_kernel_spmd", file=sys.stderr)
        per_row = _run_spmd(x16)
    # per_row[c*128 + p, r] = loss of global row c*1024 + r*128 + p; the mean
    # over all entries is permutation-invariant.
    out = np.float32(per_row.mean(dtype=np.float64))
    if _want_timing:
        return np.asarray(out), None, per_row
    return np.asarray(out)


if __name__ == "__main__":
    rng = np.random.default_rng(0)
    x = rng.standard_normal((B, 2, D)).astype(np.float32)
    print(kernel(x))


# revision 9
# speedup vs baseline: 1.4534x; 1.4534x over previous
"""Trainium2 Bass kernel for nn_LossFunction_103079215159 (triplet-style loss
with online hard-negative mining).

Math (B=8192 rows, D=256 features; x[:,0]=anchors x0, x[:,1]=positives x1):
  a = l2norm(x0), p = l2norm(x1)
  dist[i,j] = || a_i - p_j + eps ||  (via gemm expansion), diag masked +inf
  top5 smallest per row -> pick rank[i]-th (RNG-derived, data-independent)
  loss = mean relu(||a_i-p_i+eps||^2 - ||a_i-p_neg+eps||^2)

Reduction used: with s[i,j] = <x0_i, x1_j/||x1_j||> (raw-anchor units),
  d2[i,j] = 2 - 2*s[i,j]/||x0_i|| (+O(1e-6) eps terms, negligible), so
  loss_i = (2/||x0_i||) * relu(s_sel[i] - s_ii[i]) where s_sel is the
  rank[i]-th LARGEST masked s row value. Positive per-row scaling cannot
  change the row's top-k ordering, so the 2/||x0|| factor applies at the end.

Distribution: 8-way data parallel over rows. Each core receives ONLY its
[1024, 512] slab of x (fp16 over the wire), normalizes + transposes its own
positives locally, then an on-device AllGather shares the K-major scaled
positives with every core (so the full [8192,256] matrix never crosses the
host->device wire 8x). The self-match diagonal is masked by accumulating
-3e4 into the PSUM diagonal via a per-core selector input (dsel), through
the PE itself. Matmuls run in fp16 (values are O(10), fp16 has 11-bit
mantissa; rel err vs the f32 reference lands ~1e-4..1e-3, gate is 2e-2).

Host side: one fp16 cast of x (~15 ms), no other prep. The jitted 8-core
executable and the constant inputs (one-hot rank selector, diag selector)
are cached per process, so repeated kernel() calls only ship the 8.4 MB
fp16 slab and fetch the [1024, 8] per-row losses once.
"""

import base64

import numpy as np

B = 8192
D = 256
NCORES = 8
M = B // NCORES  # 1024 rows per core
RB = M // 128  # 8 row blocks per core
NG = 8  # granules of 1024 cols each
GW = 1024  # granule width

NEG_BIG = -30000.0  # diag mask; must fit fp16 and dominate |s| <= ~50

# rank[i] in {0..4}: which of the 5 nearest negatives to use per row.
# Reproduces exactly (verified):
#   k1, k2 = jax.random.split(jax.random.key(1))
#   coin = jax.random.uniform(k1, (8192,)) < 0.5
#   rank = jnp.where(coin, 0, jax.random.randint(k2, (8192,), 0, 5))
_RANK_B64 = (
    "AAIEAAAAAAAAAAIAAwAAAAAAAAAAAAMAAAIAAAMABAAAAAAAAwACAAABAAQCBAADAAACAgAEAwAC"
    "AAMEAAAAAwEEAQMAAAIAAgAAAAAAAAAEAAQAAwAABAECAAIAAAAAAgADAAACAwQABAAAAgMAAgAE"
    "AwAAAgACAAECAAEAAAECAQEBAAAABAACBAAAAAAAAAEAAAAEAQAAAAIAAgADAAEAAAAAAQAAAQME"
    "AgAAAAEEAAAAAAMAAQAAAAAEAAAEAQAAAAAAAAAAAAAAAAADAQQAAAAAAgABAAAAAAADAAADAAQA"
    "AAAAAwMAAAAEAAAAAAAAAAEAAAMAAAAAAAQAAAACAgAEAQAAAAABAAADAgABAAIAAAAAAwQCAAAD"
    "AgAAAAADAgAAAQAABAAABAAAAAAAAAIAAAEABAADAAAAAAAEAAAAAQEBAAAAAAMAAAIAAAAAAAMA"
    "AwIDAAEAAQQAAAIAAAEEAAECAAIAAAEAAAADAAIAAQICAAABAgAAAQAAAAIAAAADAAEDBAAAAQEA"
    "AgAAAAAEBAAAAAEAAgECAAIEAAAABAAEAQIABAAAAAAAAAAAAAMBAQAAAAMCAgADAAIDAwQDBAAE"
    "AAAAAAAAAAEAAAEAAwMAAAAAAAAAAAABAAAAAAAAAAEAAAADAgMAAAMAAAAAAAMAAQAAAAAAAgAA"
    "BAAAAAMBAQABAAAAAAAAAAIAAwAAAgAEAwABAAAAAAAAAAAAAAIAAgABAgAEAAABAQIAAgIDAgAE"
    "AAAAAAAAAQAABAAEAAAAAAAAAQIAAgAAAAMAAQACAAAAAAADAAQAAQABBAAEAAMABAABAQADAQAA"
    "AgABAgAEAAIAAAAAAgAAAwAAAwAAAAAEAAAAAAEAAAAAAAIEAAAAAgAABAEAAgAAAAAAAAEAAAAC"
    "AAECBAADAAAAAQAAAAIAAAAAAgMAAAAAAQAAAAQAAAAAAAMEAwEAAgEAAAAAAAAABAADAQIDAAAA"
    "AAEAAwAAAgAAAAEAAgAAAAAAAgAAAAAABAAEAAACAAIAAAQAAgADAAEAAAQAAAACAAECAwIEAAAA"
    "BAQAAAQABAMAAAQAAwIAAQMAAAQAAAACAAAEAAAABAAAAAAAAAMBAAEAAAQDAAAAAAQDAAAAAAIA"
    "AAAEAwACAAQAAgACAAACAQQAAAQDAgQDAQAAAAAEAAADBAECBAAEAAEBAAAAAAEAAgAAAwAAAgAB"
    "AwAAAgAEBAAAAAIEAAAAAwACAAIBAAABAwQAAQAAAAQAAAAAAAIAAAEBAAIAAAAAAAEAAAAAAAEB"
    "AAAAAgACAAAAAAMAAwAAAAAABAMABAMAAQQBAAQCAAEDAAAAAAIAAAAEAAMDAAAEAAEAAQAAAAAA"
    "AAICBAABAQQEAAAAAAQAAQABAAEEAAACBAAAAAMAAAAABAAAAAEBAAICAAIAAAAAAAAEBAAAAAMC"
    "AAQDAAABAAQCAAEAAAAABAQEAAIBAAAAAgAEAAEAAAIEBAACAAIAAAAABAMDBAQAAAAAAAIAAgAA"
    "AAACAAABAwMDAAAAAAAAAAACAQAAAwAAAAAEAAAAAAMAAAAAAgMAAAICAAMAAAAEAAAAAAABAAAA"
    "AAABAAAAAAMAAAEEAAIDAAEBAAQAAAMCAAAAAAAEAAACAAMAAAACAwAAAwAEAAAAAAQAAwABAAAC"
    "AwAAAAEABAQBAAIAAAIAAwAEAAEAAAACAgAAAAEEAAQAAAADAAMDAAQDBAABBAACAwAAAAAEAAMA"
    "AgQABAIAAAAEAAQCAQMAAAIBAAIAAAQEAAACAAEAAAAAAAEAAAABAAEAAAAABAAAAAAABAADAAAA"
    "BAABBAABAAADAAAAAAAAAAAAAQAAAAAAAAMAAQAAAQACAAAAAAACAAMAAAMAAwIBAAAABAAAAAMA"
    "AAAAAAABAAABAQIBAAAAAgAAAAAEAAAAAAQAAAAAAwAAAAAAAgAAAAAAAAAAAAACAgAAAAABBAAA"
    "AwACAAEDAAAAAAQAAQACAAAEAAAAAgAAAAIAAAMBAAAAAAIEAwAAAAQAAAMAAAMAAAAAAAAAAAMC"
    "BAQAAAMAAAEBAQAAAAAAAAIAAAMAAAMAAAAAAAIABAAAAAABAgAAAAAEAAQCAAIAAAIDAAMBAAAA"
    "AwAAAQADAwABAAADAAAEAwAAAAAABAMAAAEAAAAAAAAAAAAAAAAAAAAAAAACAAAAAAICAgACAAMA"
    "AAACAwAAAAIAAQAAAAAEAQAAAgAEAAEAAwAEAAAAAAAAAAQAAwAAAwAAAAQEAgAAAAMEAAAAAAAB"
    "AwQAAgADAgEDAAQDAAAAAAIAAAAAAAAAAAAABAQAAAEEBAABAAAAAQQAAAAABAAAAAMCAAAAAAAD"
    "BAAAAAEEAwIAAAADAAAAAAAEAAIAAAMBAAADAAAAAAAAAgAAAAMCAAAEAgACAAADAAAAAwABBAAD"
    "AAIAAAAAAQAABAADAAAAAAQAAQABAAMAAwADAAAAAAAAAAMEAwADAwQBAAAAAAMAAAAAAAEDAAAE"
    "AQAAAAAAAgAAAQAAAAICAAIEAAABBAACAAABAgAAAQAABAIDAgAEAAMAAAAAAAEEAAMDBAADBAAA"
    "BAAAAAADAAABAwADAAAAAAMAAAQAAQIAAAAAAwICAAIAAAIAAAAAAQAAAAICAAMAAAEAAgQAAAAA"
    "AAQAAAAABAAAAAEAAAIAAAAAAAAAAAAAAAMABAAAAAADAgAAAAAABAAABAAAAwICAAIAAAACBAAD"
    "AAAAAAADAAABAAAAAQAAAAACAgAEAAAAAAAEBAAAAAAAAAIABAQBAAAAAAAEAQAAAAIAAQADAAAD"
    "BAADAAAEBAQAAAACAAAEAAAEAAAEAAIBAAAAAgECAAAAAAMCAAIEAgADAAMAAAADAAEAAQAAAAAB"
    "BAADAQAAAAAAAQADAAAEBAIAAAIAAQIDAAACAwAAAAMAAAAAAAAAAAQABAMAAAIDAAABAgEAAAAB"
    "AAEBAAIEAwAABAACAAQAAwEAAAAAAAAAAAABAQAAAAMBBAMAAwQABAMABAAAAwMDAQQEAAABAAEB"
    "BAAAAAAAAAABAAEDAQQAAAAABAICAAIEAAMAAAAAAwADAAQDAAECAQAAAAAAAAAAAAMCAgAAAAIA"
    "AAQEAAAAAAEAAAAAAgEAAQQAAAAEBAQDBAICAAADAgIAAQAAAQABAgQCAAABAwAAAwABAAQDAAAA"
    "AAAEAAAAAgABAAAABAAABAAAAAAAAwAEAAAAAAMAAwAAAAAAAAABAAAAAwMAAQMAAAAAAgABAAAA"
    "AAMAAQAAAQACBAAAAQAAAAECAgMAAAAAAAMAAAAEAgAAAwQCAAIAAAIAAAAAAAADBAAAAQAAAAAA"
    "AAEEAAAAAAAAAgQAAAADAAADAAAAAAAAAAAAAAIBAAEEBAAAAAAEAAAAAwABAAIBAwAAAAMEAAAA"
    "AgIDBAMAAAABAAEAAAMBAAMCAAAAAAADAAIBAAADAAAAAAABAQAAAAIAAAAEAAEAAAAAAAAABAAE"
    "AAAAAAMAAgEAAQMAAAAAAAACAAMBAgABAwAAAAAEBAAAAQADAAEAAAMBAAAAAQIAAwABAgECAQMA"
    "AAAAAAACAAAAAAEAAAAAAAAEAAAAAAMEAwABAAAEAAAAAAAAAAECAQEAAAAAAAAAAAACAAAAAQAE"
    "AAQAAAACAAQAAAAAAAAAAAEAAAABAAQBAwIAAAAAAAQCAAEBAAIAAgAAAAMEAAAEAAACAQEAAAAA"
    "AAAAAAQAAQQCAAQEAgMDAAQAAAMAAAADAAAEAAEAAwAEBAQDAAACAAEAAAAABAMDAAMAAAEAAAQA"
    "AgMAAwAABAABAAIDAAQAAAICAAIAAAAAAAIEAgAAAgAEAwIAAAABAAAEAQAAAwAAAAACBAECAQAA"
    "AwAAAwQAAwQDAAAAAAACAQQDAAAAAAAEAAAAAwMBAAAAAAQAAAAAAgIAAAADBAADBAAEAAQABAAA"
    "BAAAAwQBAAAAAAACAAACAAIAAAAEAAEABAAAAgAAAAAAAAAAAAEEAAAAAwAAAQIAAAMAAQACAwQE"
    "AQABAwAAAAAAAAAAAAMBAAAABAIAAAAAAAIEAAAAAgAAAwAEAwADAAACAAEDAwQEAwAAAAAAAAAD"
    "AwACAAIDBAAABAAEAAAAAAACAgACAgICAAAAAAAAAAADAAIDAAQBAAMAAgAAAgAAAAAAAAAAAQAE"
    "AwQAAQAAAAIBAgAAAAEAAAQAAAAAAAIAAAABAQAAAwABBAADAwABAAIAAAAAAQQBAgIABAAAAAQC"
    "AAACAgMCAwQDAAAAAAACAAABAAICAAAAAgIAAAAAAQIAAAAAAAABAAAAAAAAAAAAAAIBBAQEAAQA"
    "AgQBAAEAAAAAAAAEAwAAAAAABAAAAQABAAAAAgAAAAEAAAMBAgMAAQAAAQAAAAQAAAQAAAAAAAAA"
    "AAEAAgIAAAIAAAAAAAAEAgAAAAIBAAAAAAAAAAIEAAAAAgIAAAQAAAAAAwAAAgIAAAIABAMAAQAA"
    "AAAAAAADAAAAAAAAAAADAQADBAAAAwAAAAAAAAABBAACAQAAAAABAgADAAAAAAAAAgADAAMAAAID"
    "AAIAAAAEAAAABAAAAAAAAwABAQECAwAAAAEAAAAAAAQAAAAAAAEEAAMAAAAEAAAAAAIAAwECAAAA"
    "AQAAAAABAAAAAAAABAAAAAQABAECAAIBAAECAAAAAAADAAACAgAEAAQAAAAAAAMABAAAAQEABAAA"
    "BAEAAwMEAAMAAAQABAQDBAAAAAAAAwAAAgEEAAABAAAAAAAAAAIDAgAEAQABAwACAAAEAQQEAAIA"
    "AAADAAABAgMEBAAAAAAAAgACAAAABAQAAAABAAAAAAMDAwEAAAAEAAMABAAEAwIAAAQAAQAEAAAA"
    "AgAAAAAAAAEAAAAAAAAAAwEAAAEAAgACAAAAAQADAAAAAAEAAAAAAAAABAECAAAAAAIAAAQBAgIA"
    "AwAAAAIAAAMAAAAEAAIAAAIAAQACAAAAAAAAAAAAAAMCAAADAAEBAgAAAwAAAwADAwADAAQAAAAA"
    "AAIBAwAAAQAAAAEAAAABAAAAAAAEAAEAAAQAAgQDAgEEAgMCBAAAAQIAAgAAAgIAAAABAAQAAAAA"
    "AAAAAAEAAAAAAwQAAAAAAwAEAAAAAAADAAAAAAAEAAABBAAAAAAAAwQEAAAAAgQAAAAEAgAAAAAA"
    "AAEAAAECAAAABAIEAAAAAgAAAAECAgAAAAMDAgAAAAIBAAAEAAAAAAAAAAQAAAMAAAAAAwAAAQQA"
    "AAEDAQADAAMAAAAAAAAAAAEAAAIEAAICAQAAAAIAAAAAAAEBAAEAAAAAAAACAAMDAAEAAQAAAAAA"
    "AAADAAADAAAAAAEBAwMBAwEAAAIBAAQAAAAAAAADAAAAAAEAAAMAAAABAwMAAAAAAwAABAAAAAAA"
    "AwIAAAIDBAAEAAAAAwIAAgAAAAAAAAAAAAIAAAAAAwADAAMABAMAAgQAAwAAAwAAAAAEAgADAQAE"
    "AAQAAgAEAAAAAAADAAMAAAADAgACAQQAAAAEAAEABAAAAwEABAABAgAEBAABAwMEAAAAAQAEAgEE"
    "AAMBAAAAAAAAAAAEAAAAAAEAAAABAAAAAwAAAQIAAAMAAAAAAAAAAAAAAAACAAACBAACAAAAAAIA"
    "AAICAAEAAQAAAwMAAwEBAwAEAAMDAAQCAAIEAAABBAABBAEEAAECAQMEAAAAAAACAwADBAIBAwAB"
    "AAAAAwACAgMCAAMAAAAAAwMAAAQAAAQAAQAAAAAAAAMABAQAAwAAAAEAAgABAAAABAEAAAAAAAAC"
    "AQIAAAAAAAMAAwIAAQACAQMEAwQAAAAEAAMAAQAAAAADAQABAAQAAAABAQMBAAAEAQAAAAAAAAAE"
    "AAAAAAIEAAAEAAAAAAAEAwEAAAAAAAIAAgAAAwEAAAEAAgAAAAMAAAQEAwAAAAADAQABAwAAAAAB"
    "AwADBAAEAQAAAwAABAAABAAAAAAAAAABAAAAAAMCAAAAAgEAAAQDAQAAAAMDAAAEAAIABAAAAAAA"
    "AQMEAAAAAAAAAAAAAAEEBAAEAAQDAAAAAAAAAgAAAAMAAwAAAAEAAAAAAgAAAQAAAgAEAAADBAAA"
    "AwABAAAAAwADAAICAAIAAAICAgMEAgAAAAAAAQACAAQBBAAAAQEBAAAAAAIAAAAAAgACAAIAAAAA"
    "AQAABAIDAAAAAAAAAAAAAAAEAAAAAAABAQAAAAAEAAAAAwABAwAAAAIEAAAABAEAAgMCAwACAAAC"
    "AAADAAAAAwAAAAMAAwMAAgACAAAAAAEDBAQAAwIDAAAAAAQCAgADAAADAgAAAAAAAwAAAAMBAQEA"
    "AwEAAwABAAAAAAMCAAAAAAADAAAABAQDBAAABAEAAwAAAAQEAAAAAwAAAgIBBAACAAABAAQAAAAD"
    "AAQABAICAAAEAQMAAAACBAEAAAIAAAMEAAAABAADAAAAAAIAAAMAAQAAAAABAAIAAAACAwMDAAAA"
    "AgACAAIEAAAAAAEEAAEAAAMDAAQEBAEAAAAAAAAAAAEAAgAEAAQAAAAEAAMABAABAQMAAQADAAID"
    "AAAAAAMCAgEAAwQAAgIAAAAEAAEAAAAAAAAABAAAAAAAAAQAAAAEAAAABAAAAAAAAAAAAAAAAAAA"
    "AAAEAwMAAQMAAwQAAQABAwACAAMAAAAAAAADAQAEAgAAAgIBAAQBBAAAAAAAAAQAAQAEAgAEAAIC"
    "AAIEAAIAAgAAAAADAAAABAQAAAACBAEEAwIABAACAAAAAAMABAABAAAAAAMAAAQAAAABAAMAAAAA"
    "AgACAAMAAAAAAwAAAAIAAAAAAAAAAAMEAAQEAAIAAQAAAAQDBAAAAAQABAMAAQQAAQAAAAEEAAMD"
    "AQAABAADAAAAAAABAgAAAAAABAIAAAABAAAABAABAgECAwMAAAACAgEABAABAAAAAgEBAAAEBAAC"
    "AAAAAgEAAAMAAAACAAAAAgMAAAAAAAQBAAAAAAACAQMCAAABAAADAAADAwABAAIAAAADAAADAQAA"
    "AAAABAACAAAAAAIAAAAABAMDBAQAAAAAAAQBAAQAAAAAAAAAAQAAAAEEAAMABAEAAAAEAgAAAAMA"
    "AAAAAgMCAgIAAAAAAgAAAAAAAAMAAAAAAAEAAAAAAgMBAAMAAAAABAMEAAQAAAMAAwACBAAEAAAB"
    "AAAAAAACBAQABAAEAgQAAAAEAQMDAAMAAAIEAQAEBAADAQIABAEDAAAAAgQABAADAAAAAgACBAMB"
    "AAMDAAAAAAAAAAIDAAAAAAIABAADAAAAAQAAAAAAAAAEAQAAAgABAAMDBAIBAAAABAADAAMEAwQA"
    "AAQCAAEAAwMAAAQBAAACAAABAAEAAAQCBAMBAgAAAAAAAAAABAQCAwMABAAAAAAAAAAAAAAAAQME"
    "AAAAAQAABAACAAMCAwEBAAACAgAAAgEAAAADAAAEBAAAAAAAAAABAAABAwMAAAMCAwAEAwIAAAQA"
    "BAICAAEBAAIAAAACAgIBAAAAAgQCAgAAAQQAAAAAAAAAAAMEAAADAwQABAACBAQAAwQAAQEDAQAA"
    "BAAAAAAAAwAAAAACAAMAAgMEAwEAAAAAAAEDAAAAAAIBAAQAAAMAAAMABAAEAAEEAwMAAAABBAAE"
    "AAIEAwAAAAAAAAMAAgQAAAMAAAEAAQIAAAMDBAAABAAAAAMAAAAEAAAEAAMAAAAAAAAAAAMAAAAE"
    "AAABAwAAAQAAAAEEAAAAAAIAAQAEAAAAAAADAAMAAAQDAAAAAgQCAgEAAAIBAAAAAAADBAIAAAMA"
    "AAQAAQQAAAACAAAAAAMAAgAAAQMAAAAAAQADAAIAAAAAAgAABAAAAAQEBAAEAQQAAwABAAACAAAA"
    "AAAAAAAAAAADAAAEAAABAgADAAIAAgEDAAADAAAAAAADAwQAAAMBAAAAAAAAAAAAAgABAQADAQQA"
    "BAAAAwAAAAABAAAAAAIDAAAAAwAEAAAAAQAAAAAAAwAAAAIDAAAAAwADAAQAAAEAAAECAAIABAAA"
    "BAAABAACAAMAAQAAAAIAAgIAAgAAAAQAAQACAAACAAABAAEBAAIDAAIABAAAAwEAAgMAAAAAAAMA"
    "BAACBAAAAAAABAABBAAEAAAAAQQAAQAAAAAEAgAAAAAAAwADAAAAAAAAAAMAAAAAAAEAAAAABAEA"
    "AAAEAgIAAAIAAAAAAAAAAAAAAAEEAAADAAAAAAEAAwAAAAMEAgAAAAAAAAAAAAIEAAEAAQAABAAA"
    "BAEAAAQAAwAAAwABAAIDAwQEAAAAAwQAAAQABAMAAAECAgACAAIDAAAAAQIEAAQABAQDAAAAAAAA"
    "AAAAAAAAAwABAwAAAQADAwIAAAAAAQABAAAAAAEABAQBAwABAAADAgAEAAIAAAMABAEAAAEAAQAA"
    "BAMAAwQCAwMAAQMCAwQAAwAAAAEABAAAAAEAAgEAAAAAAAAAAAAAAAAAAgAEAQAAAAEAAAAEAwAA"
    "AQIABAMEAAABAAMAAgEEAAIAAAEEAAABAAABAQAAAAAAAgIAAAAAAAADAgABBAMEAgACBAACBAQA"
    "AgADAAACAgQAAwADAwAEBAQAAAEBAAAABAECAAAAAAAABAACAAAEBAAAAAADAAAEAAMAAAIBAAAA"
    "AAQAAQAABAAAAAACAAEDAwAEBAAAAAAAAAACAQAAAAAEAAIAAAADAAAAAAIAAwAAAAEEBAAAAgAD"
    "AAAAAgEAAAQAAAEAAAAAAAIEAAMAAwQABAACAAEBAAEAAAEABAAAAAICBAQAAQAAAgIEAAAAAAAA"
    "AAAAAAAABAIBAAAAAgIAAAACAQAAAAABAAAAAAQEAgAEAAABAAAAAAAAAAEAAAMCAwAEBAMDBAAA"
    "AAABAAABAAEBAAABAwAAAAABAAABAwMAAAABAAMEAAAAAgAAAAQAAAACAAMAAAAAAAAAAAQAAAQD"
    "AAAABAABAAIAAAIAAAAAAAICAwACAwABAAAAAAQAAwADAgAAAAAAAgEABAIAAAAAAAABBAAAAAIC"
    "AAQAAAQAAAEAAwMDAAAAAQAEBAAAAAEAAAEBAAAAAgAAAwIABAADAAAEAgAAAAAABAAAAAAAAAAC"
    "AAQAAgAEAwAAAAAEAAMEBAEAAQACAAAEAAAABAAAAAAAAAAEAQQAAAQEAAQAAgAAAQEAAQAAAAQE"
    "AAABAAAAAAQABAAEAQAABAACAwACBAQEAAAAAQEAAQABAAAAAAAAAAAAAQAAAQAAAAAEAAACAAAA"
    "BAACAAEAAAAAAAMAAAIAAAMEAQAAAAIBAAIBAAAABAECAAAAAAAAAAABAAMBAAAAAwQAAgAAAwAA"
    "AwAEAQQAAwAAAQQAAwQAAAABAAABAAAEAAQAAAACAAABAAAAAAAAAQIAAAABAAAAAAICAAACAAIA"
    "AAADAgMCAAABAAAAAwACAAMABAAAAAAAAAAAAAAAAAIAAAAAAAQBAAAAAAECAQMBAAAAAAACAAAD"
    "AAAAAAQCAAQBAAACAAAAAAMAAwIAAgMAAAABAwMDBAAABAAEAAAAAAEBAAQCAQAEAAQABAIAAAID"
    "AAEAAQAAAAACAAQAAAABAAADAQECAAAAAAQAAAMABAACAAAAAAQAAAAAAAAAAQEDAAABAwQDAwIA"
    "BAAAAQADAAAAAgAEAwAABAABAQAABAABAAQAAgAAAAAAAAQAAAMBAAACBAAEAAEEAAAABAAABAAA"
    "AAAABAMDAAEBAAAAAAAEAgMAAAAEAgADAAACAgAAAAMAAAQBAQAAAQAEAgAAAAMDAAAAAAABBAAA"
    "AAAAAwQBAAIAAAABAAIAAAIABAMAAAAEAwMAAAABAAAAAwECBAAABAAAAAACAAAAAAAAAAAEAQIB"
    "AAAABAMAAAQCAwEBAgAAAAQAAQAAAAABAAAAAAIAAwACAwECAQAAAgMCAwAEAAAEAQQAAAAAAwAA"
    "AAMAAAMAAAAABAAAAAAAAAMAAAMEAAAAAAAEAAAAAAAAAAQAAwECAAQAAAAAAgAAAAAAAAAAAAAA"
    "AAAEAAADAwAAAAMCAAIAAAAAAwAAAgADAAACAAADAAAAAAMBAAEBAAECAAADAAAEAQMDBAACAAAC"
    "AAABAAACAAQAAAAAAQAAAAAAAQABAwQAAAQCAAAAAwMAAQADAAMAAAMAAAIAAAAAAAAAAAEEAAAA"
    "AAMAAAMEAAACAAAAAAMAAwIAAQMAAgIAAAIAAQAAAAAABAMAAAAAAgEAAAABAQEBAAQAAgQDAAAA"
    "BAMAAAEAAAAAAgIAAwMAAAAABAIAAAADAAECAgIAAAEBAAMBAAQAAgAAAAIAAAIAAAAAAAQEAAAD"
    "AQEEAQIDAAACAAACAAIEAAECAAAAAgMCAwACAAABAwAAAwAAAAAABAAEAAQDAAAAAAABAQEBAAAE"
    "AAAAAwAAAgAAAAADAAECAQMAAAABAAACAAAAAAAAAwMAAAIAAAIAAAEBAAIEAAAEAAAAAAAAAAMA"
    "AQQAAAMEAAMAAwMAAQAAAAAAAAMEAAQCAAIDAAMDBAQAAAAEAAEAAAMCAQACAgAAAAEDAAQAAwAA"
    "AAAAAQQAAAICBAMAAAEAAAAAAAQDAAAAAQAAAQADAAADAAAAAAAAAQAABAAAAAAAAQADAgICAQIA"
    "AAIBAAEAAwAAAAAAAAADAwAAAAAABAIAAAAAAAAEAAMABAAAAAAAAAQAAwQABAAAAAAAAAAAAwED"
    "AAMAAAAAAAAABAMAAAAAAwEAAgABAAAAAQAAAAACAAAAAAAEAQABAAABAQAAAQAAAAMAAgABAAMA"
    "AAAABAAEAQAAAAMABAAAAAEAAQAAAwQDAAACAAQEAAACAAAEBAAAAAMBAAABAAACAAAAAAQAAAAB"
    "AAADAQIBAAADAAEAAQAAAgMBAAADAAIDAAQAAAAAAQEBAQAAAgMAAAACAAAEAwABAAAAAAAEAAAD"
    "AAEEAwEAAQAAAQACAAEAAAMAAQMAAgAAAAIAAAQAAAAAAAIDAAAAAAA="
)


def _rank_to_b64():
    """(debug helper) regenerate _RANK_B64 with jax on CPU."""
    import jax
    import jax.numpy as jnp

    cpu = jax.devices("cpu")[0]
    with jax.default_device(cpu):
        k1, k2 = jax.random.split(jax.random.key(1))
        coin = jax.random.uniform(k1, (B,)) < 0.5
        rank = jnp.where(coin, 0, jax.random.randint(k2, (B,), 0, 5))
    return base64.b64encode(np.asarray(rank, dtype=np.uint8).tobytes()).decode()


_RANK_CACHE = None


def _get_rank() -> np.ndarray:
    """rank[i]: which of the 5 nearest negatives the reference picks per row.

    Must reproduce the reference's jax.random draws bit-exactly. The default
    PRNG impl here is "rbg", whose output is backend-dependent, so compute on
    the CPU backend (the grading reference runs on CPU). Falls back to the
    embedded constant (generated the same way) if jax is unavailable.
    """
    global _RANK_CACHE
    if _RANK_CACHE is not None:
        return _RANK_CACHE
    try:
        import jax
        import jax.numpy as jnp

        cpu = jax.devices("cpu")[0]
        with jax.default_device(cpu):
            k1, k2 = jax.random.split(jax.random.key(1))
            coin = jax.random.uniform(k1, (B,)) < 0.5
            rank = jnp.where(coin, 0, jax.random.randint(k2, (B,), 0, 5))
            r = np.asarray(jax.device_get(rank)).astype(np.uint8)
    except Exception:
        r = np.frombuffer(base64.b64decode(_RANK_B64), dtype=np.uint8)
    assert r.shape == (B,)
    _RANK_CACHE = r
    return r


_NC_CACHE = None


def _build_nc():
    import concourse.mybir as mybir
    import concourse.tile as tile
    from concourse import bacc
    from concourse.masks import make_identity

    F32 = mybir.dt.float32
    F16 = mybir.dt.float16
    F8 = mybir.dt.float8e4
    AF = mybir.ActivationFunctionType

    nc = bacc.Bacc()
    xs = nc.dram_tensor("xs", [M, 2 * D], F8, kind="ExternalInput").ap()
    oh = nc.dram_tensor("oh", [M, 8], F32, kind="ExternalInput").ap()
    dsel = nc.dram_tensor("dsel", [128, NG], F32, kind="ExternalInput").ap()
    loss = nc.dram_tensor("loss", [128, RB], F32, kind="ExternalOutput").ap()

    with tile.TileContext(nc) as tc:
        with (
            tc.tile_pool(name="const", bufs=1) as constp,
            tc.tile_pool(name="big", bufs=1) as bigp,
            tc.tile_pool(name="small", bufs=4) as smallp,
            tc.tile_pool(name="pst", bufs=2, space="PSUM") as pst,
            tc.tile_pool(name="psg", bufs=3, space="PSUM") as psg,
            tc.tile_pool(name="dram", bufs=1, space="DRAM") as dramp,
        ):
            # ---------------- constants ----------------
            identf = constp.tile([128, 128], F32)
            make_identity(nc, identf)
            ident16 = constp.tile([128, 128], F16)
            nc.scalar.copy(ident16, identf)
            dsel_sb = constp.tile([128, NG], F32)
            nc.scalar.dma_start(dsel_sb, dsel)
            oh_sb = constp.tile([128, RB * 8], F32)
            nc.scalar.dma_start(
                oh_sb.rearrange("p (r k) -> p r k", r=RB),
                oh.rearrange("(r p) k -> p r k", p=128),
            )
            # negsel[:, g*128:(g+1)*128] = I * dsel[g]  (NEG_BIG iff g == my core)
            negf = constp.tile([128, NG * 128], F32)
            negsel = constp.tile([128, NG * 128], F16)
            for g in range(NG):
                nc.gpsimd.tensor_scalar_mul(
                    negf[:, g * 128 : (g + 1) * 128], identf, dsel_sb[:, g : g + 1]
                )
            nc.scalar.copy(negsel, negf)

            # ---------------- slab load ----------------
            # fp8 over the wire; upcast once to fp16, then the rest of the
            # pipeline is identical to the fp16 version.
            # xs_sb[:, r*512 + 0:256]   = anchor rows block r (fp16)
            # xs_sb[:, r*512 + 256:512] = positive rows block r (fp16)
            xs8 = bigp.tile([128, RB * 2 * D], F8)
            nc.sync.dma_start(
                xs8.rearrange("p (r d) -> p r d", r=RB),
                xs.rearrange("(r p) d -> p r d", p=128),
            )
            xs_sb = bigp.tile([128, RB * 2 * D], F16)
            nc.vector.tensor_copy(xs_sb, xs8)

            # ---------------- local positives: norm + scale ----------------
            np2 = constp.tile([128, RB], F32)
            nps = constp.tile([128, RB], F32)
            invnp = constp.tile([128, RB], F32)
            sq = smallp.tile([128, D], F32, tag="sq")
            for r in range(RB):
                nc.scalar.activation(
                    sq,
                    xs_sb[:, r * 512 + 256 : r * 512 + 512],
                    AF.Square,
                    accum_out=np2[:, r : r + 1],
                )
            nc.scalar.activation(nps, np2, AF.Sqrt)
            nc.vector.reciprocal(invnp, nps)
            ps16 = bigp.tile([128, RB * D], F16)  # unit positives, fp16
            for r in range(RB):
                nc.gpsimd.tensor_scalar_mul(
                    ps16[:, r * D : (r + 1) * D],
                    xs_sb[:, r * 512 + 256 : r * 512 + 512],
                    invnp[:, r : r + 1],
                )

            # ---------------- local transposes (K-major operands) -----------
            # psl: [k-chunk, col] layout of local scaled positives ([128, 2*M])
            psl = bigp.tile([128, 2 * M], F16)
            for k in range(2):
                for r4 in range(2):
                    pt = pst.tile([128, 512], F16)
                    for j in range(4):
                        r = r4 * 4 + j
                        nc.tensor.transpose(
                            pt[:, j * 128 : (j + 1) * 128],
                            ps16[:, r * D + k * 128 : r * D + k * 128 + 128],
                            ident16,
                        )
                    nc.scalar.copy(
                        psl[:, k * M + r4 * 512 : k * M + (r4 + 1) * 512], pt
                    )
            aT = [bigp.tile([128, M], F16, name=f"aT{k}") for k in range(2)]
            for k in range(2):
                for r4 in range(2):
                    pt = pst.tile([128, 512], F16)
                    for j in range(4):
                        r = r4 * 4 + j
                        nc.tensor.transpose(
                            pt[:, j * 128 : (j + 1) * 128],
                            xs_sb[:, r * 512 + k * 128 : r * 512 + k * 128 + 128],
                            ident16,
                        )
                    nc.scalar.copy(aT[k][:, r4 * 512 : (r4 + 1) * 512], pt)

            # ---------------- s_ii and anchor norms (pre-CC, off-path) ------
            # s_ii = <a_raw16, p_unit16>: same operand bits as the matmul path.
            sii = constp.tile([128, RB], F32)
            for r in range(RB):
                dot = smallp.tile([128, D], F32, tag="dot")
                nc.gpsimd.tensor_mul(
                    dot,
                    xs_sb[:, r * 512 : r * 512 + 256],
                    ps16[:, r * D : (r + 1) * D],
                )
                nc.vector.reduce_sum(sii[:, r : r + 1], dot, axis=mybir.AxisListType.X)
            na2 = constp.tile([128, RB], F32)
            na_half = constp.tile([128, RB], F32)
            inv2na = constp.tile([128, RB], F32)
            sqa = smallp.tile([128, D], F32, tag="sqa")
            for r in range(RB):
                nc.scalar.activation(
                    sqa,
                    xs_sb[:, r * 512 : r * 512 + 256],
                    AF.Square,
                    accum_out=na2[:, r : r + 1],
                )
            nc.scalar.activation(na_half, na2, AF.Sqrt, scale=0.25)
            nc.vector.reciprocal(inv2na, na_half)

            # ---------------- AllGather the K-major positives ----------------
            pslab_d = dramp.tile([2, 128, M], F16)
            nc.sync.dma_start(
                pslab_d.rearrange("k p m -> p k m"),
                psl.rearrange("p (k m) -> p k m", k=2),
            )
            pall_d = nc.dram_tensor(
                "pall_d", [NG, 2, 128, M], F16, addr_space="Shared"
            ).ap()
            nc.gpsimd.collective_compute(
                "AllGather",
                mybir.AluOpType.bypass,
                replica_groups=[list(range(NCORES))],
                ins=[pslab_d.opt()],
                outs=[pall_d.opt()],
            )
            pT = [bigp.tile([128, B], F16, name=f"pT{k}") for k in range(2)]
            for g in range(NG):
                for k in range(2):
                    nc.sync.dma_start(pT[k][:, g * M : (g + 1) * M], pall_d[g, k])

            # ---------------- main loop: matmul granules + top-8 ------------
            cand = [
                constp.tile([128, NG * 8], F32, name=f"cand{r}") for r in range(RB)
            ]
            for g in range(NG):
                for r in range(RB):
                    gt = psg.tile([128, GW], F32)
                    hd = r // 4  # 512-col half holding this row block's diagonal
                    for h in range(2):
                        for k in range(2):
                            nc.tensor.matmul(
                                gt[:, h * 512 : (h + 1) * 512],
                                aT[k][:, r * 128 : (r + 1) * 128],
                                pT[k][:, g * GW + h * 512 : g * GW + (h + 1) * 512],
                                start=(k == 0),
                                stop=(k == 1 and h != hd),
                            )
                        if h == hd:
                            # accumulate dsel[g] * I at the self-match block:
                            # NEG_BIG on the diagonal iff granule g is mine.
                            nc.tensor.matmul(
                                gt[:, r * 128 : (r + 1) * 128],
                                negsel[:, g * 128 : (g + 1) * 128],
                                ident16,
                                start=False,
                                stop=True,
                            )
                    nc.vector.max(out=cand[r][:, g * 8 : (g + 1) * 8], in_=gt)

            # ---------------- epilogue: merge, select, loss -----------------
            top8a = constp.tile([128, RB * 8], F32)
            for r in range(RB):
                nc.vector.max(out=top8a[:, r * 8 : (r + 1) * 8], in_=cand[r])
            sel_all = constp.tile([128, RB * 8], F32)
            nc.vector.tensor_mul(sel_all, top8a, oh_sb)
            selv = constp.tile([128, RB], F32)
            nc.vector.reduce_sum(
                selv,
                sel_all.rearrange("p (r k) -> p r k", r=RB),
                axis=mybir.AxisListType.X,
            )
            loss_sb = constp.tile([128, RB], F32)
            nc.vector.tensor_sub(loss_sb, selv, sii)
            # fold the anchor scale in before relu: relu(c*x) = c*relu(x)
            nc.vector.tensor_mul(loss_sb, loss_sb, inv2na)
            relu_sb = constp.tile([128, RB], F32)
            nc.scalar.activation(relu_sb, loss_sb, AF.Relu)
            nc.sync.dma_start(loss, relu_sb)

    nc.compile()
    return nc


def _get_nc():
    global _NC_CACHE
    if _NC_CACHE is None:
        _NC_CACHE = _build_nc()
    return _NC_CACHE


def _host_inputs():
    """Constant (input-independent) host arrays: one-hot rank + diag selector."""
    rank = _get_rank()
    onehot = np.zeros((B, 8), dtype=np.float32)
    onehot[np.arange(B), rank] = 1.0
    dsel = np.zeros((NCORES * 128, NG), dtype=np.float32)
    for c in range(NCORES):
        dsel[c * 128 : (c + 1) * 128, c] = NEG_BIG
    return onehot, dsel


_RT_CACHE = None


def _get_rt():
    """Cached 8-core jitted executable + device-resident constant inputs.

    Mirrors bass2jax.run_bass_via_pjrt's multi-core path, but builds the
    jitted shard_map once per process (run_bass_via_pjrt re-creates the
    closure -> re-jits + re-NEFF-compiles on every call) and keeps the
    constant operands on device.
    """
    global _RT_CACHE
    if _RT_CACHE is not None:
        return _RT_CACHE

    import jax
    from jax.experimental.shard_map import shard_map
    from jax.sharding import Mesh, NamedSharding, PartitionSpec

    import concourse.mybir as mybir
    from concourse import bass2jax

    bass2jax.install_neuronx_cc_hook()
    nc = _get_nc()
    assert nc.dbg_addr is None
    partition_name = nc.partition_id_tensor.name if nc.partition_id_tensor else None

    in_names, out_names, out_avals, zero_outs = [], [], [], []
    for alloc in nc.m.functions[0].allocations:
        if not isinstance(alloc, mybir.MemoryLocationSet):
            continue
        name = alloc.memorylocations[0].name
        if alloc.kind == "ExternalInput":
            if name != partition_name:
                in_names.append(name)
        elif alloc.kind == "ExternalOutput":
            shape = tuple(alloc.tensor_shape)
            dtype = mybir.dt.np(alloc.dtype)
            out_names.append(name)
            out_avals.append(jax.core.ShapedArray(shape, dtype))
            zero_outs.append(np.zeros(shape, dtype))
    assert in_names == ["xs", "oh", "dsel"] and out_names == ["loss"], (
        in_names,
        out_names,
    )
    n_params = len(in_names)
    all_names = list(in_names) + list(out_names)
    if partition_name is not None:
        all_names.append(partition_name)
    all_names = tuple(all_names)

    def _body(*args):
        operands = list(args)
        if partition_name is not None:
            operands.append(bass2jax.partition_id_tensor())
        outs = bass2jax._bass_exec_p.bind(
            *operands,
            out_avals=tuple(out_avals),
            in_names=all_names,
            out_names=tuple(out_names),
            lowering_input_output_aliases=(),
            sim_require_finite=True,
            sim_require_nnan=True,
            nc=nc,
        )
        return tuple(outs)

    devices = jax.devices()[:NCORES]
    assert len(devices) == NCORES, f"need {NCORES} cores, got {len(devices)}"
    mesh = Mesh(np.asarray(devices), ("core",))
    spec = PartitionSpec("core")
    n_outs = len(out_names)
    donate = tuple(range(n_params, n_params + n_outs))
    sharded = jax.jit(
        shard_map(
            _body,
            mesh=mesh,
            in_specs=(spec,) * (n_params + n_outs),
            out_specs=(spec,) * n_outs,
            check_rep=False,
        ),
        donate_argnums=donate,
        keep_unused=True,
    )

    onehot, dsel = _host_inputs()
    sh = NamedSharding(mesh, spec)
    oh_dev = jax.device_put(onehot, sh)
    dsel_dev = jax.device_put(dsel, sh)
    zero_shape = (NCORES * zero_outs[0].shape[0], *zero_outs[0].shape[1:])
    zero_dtype = zero_outs[0].dtype
    in_dtype = mybir.dt.np(mybir.dt.float8e4)

    _RT_CACHE = (sharded, oh_dev, dsel_dev, zero_shape, zero_dtype, devices, sh, in_dtype)
    return _RT_CACHE


def _run_fast(x32: np.ndarray) -> np.ndarray:
    import jax

    sharded, oh_dev, dsel_dev, zero_shape, zero_dtype, devices, sh, in_dtype = _get_rt()
    # Pipeline the fp8 cast with the per-device uploads: cast slab c on the
    # host while slab c-1 is in flight (device_put is async).
    shards = []
    for c in range(NCORES):
        chunk = np.asarray(x32[c * M : (c + 1) * M], dtype=in_dtype)
        shards.append(jax.device_put(chunk, devices[c]))
    xs_dev = jax.make_array_from_single_device_arrays((B, 2 * D), sh, shards)
    out = sharded(xs_dev, oh_dev, dsel_dev, np.zeros(zero_shape, zero_dtype))
    return np.asarray(out[0])  # [NCORES*128, RB] per-row losses


def _run_spmd(x32: np.ndarray) -> np.ndarray:
    """Fallback: the stock run_bass_kernel_spmd path (re-jits per call)."""
    import concourse.mybir as mybir
    from concourse.bass_utils import run_bass_kernel_spmd

    in_dtype = mybir.dt.np(mybir.dt.float8e4)
    onehot, dsel = _host_inputs()
    in_maps = []
    for c in range(NCORES):
        in_maps.append(
            {
                "xs": np.asarray(x32[c * M : (c + 1) * M], dtype=in_dtype),
                "oh": np.ascontiguousarray(onehot[c * M : (c + 1) * M]),
                "dsel": np.ascontiguousarray(dsel[c * 128 : (c + 1) * 128]),
            }
        )
    res = run_bass_kernel_spmd(_get_nc(), in_maps, list(range(NCORES)))
    return np.concatenate([res.results[c]["loss"] for c in range(NCORES)], axis=0)


def kernel(x: np.ndarray, _want_timing: bool = False):
    """x: [8192, 2, 256] float32 -> scalar float32 loss (0-d ndarray)."""
    x = np.asarray(x)
    assert x.shape == (B, 2, D)
    x32 = np.ascontiguousarray(x.reshape(B, 2 * D))

    try:
        per_row = _run_fast(x32)
    except Exception as e:  # pragma: no cover - belt and braces
        import sys

        print(f"kernel: fast path failed ({type(e).__name__}: {e}); "
              f"falling back to run_bass_kernel_spmd", file=sys.stderr)
        per_row = _run_spmd(x32)
    # per_row[c*128 + p, r] = loss of global row c*1024 + r*128 + p; the mean
    # over all entries is permutation-invariant.
    out = np.float32(per_row.mean(dtype=np.float64))
    if _want_timing:
        return np.asarray(out), None, per_row
    return np.asarray(out)


if __name__ == "__main__":
    rng = np.random.default_rng(0)
    x = rng.standard_normal((B, 2, D)).astype(np.float32)
    print(kernel(x))


# revision 10
# speedup vs baseline: 1.8011x; 1.2392x over previous
"""Trainium2 Bass kernel for nn_LossFunction_103079215159 (triplet-style loss
with online hard-negative mining).

Math (B=8192 rows, D=256 features; x[:,0]=anchors x0, x[:,1]=positives x1):
  a = l2norm(x0), p = l2norm(x1)
  dist[i,j] = || a_i - p_j + eps ||  (via gemm expansion), diag masked +inf
  top5 smallest per row -> pick rank[i]-th (RNG-derived, data-independent)
  loss = mean relu(||a_i-p_i+eps||^2 - ||a_i-p_neg+eps||^2)

Reduction used: with s[i,j] = <x0_i, x1_j/||x1_j||> (raw-anchor units),
  d2[i,j] = 2 - 2*s[i,j]/||x0_i|| (+O(1e-6) eps terms, negligible), so
  loss_i = (2/||x0_i||) * relu(s_sel[i] - s_ii[i]) where s_sel is the
  rank[i]-th LARGEST masked s row value. Positive per-row scaling cannot
  change the row's top-k ordering, so the 2/||x0|| factor applies at the end.

Distribution: 8-way data parallel over rows. Each core receives ONLY its
[1024, 512] slab of x (fp16 over the wire), normalizes + transposes its own
positives locally, then an on-device AllGather shares the K-major scaled
positives with every core (so the full [8192,256] matrix never crosses the
host->device wire 8x). The self-match diagonal is masked by accumulating
-3e4 into the PSUM diagonal via a per-core selector input (dsel), through
the PE itself. Matmuls run in fp16 (values are O(10), fp16 has 11-bit
mantissa; rel err vs the f32 reference lands ~1e-4..1e-3, gate is 2e-2).

Host side: one fp16 cast of x (~15 ms), no other prep. The jitted 8-core
executable and the constant inputs (one-hot rank selector, diag selector)
are cached per process, so repeated kernel() calls only ship the 8.4 MB
fp16 slab and fetch the [1024, 8] per-row losses once.
"""

import base64

import numpy as np

B = 8192
D = 256
NCORES = 8
M = B // NCORES  # 1024 rows per core
RB = M // 128  # 8 row blocks per core
NG = 8  # granules of 1024 cols each
GW = 1024  # granule width

NEG_BIG = -30000.0  # diag mask; must fit fp16 and dominate |s| <= ~50

# rank[i] in {0..4}: which of the 5 nearest negatives to use per row.
# Reproduces exactly (verified):
#   k1, k2 = jax.random.split(jax.random.key(1))
#   coin = jax.random.uniform(k1, (8192,)) < 0.5
#   rank = jnp.where(coin, 0, jax.random.randint(k2, (8192,), 0, 5))
_RANK_B64 = (
    "AAIEAAAAAAAAAAIAAwAAAAAAAAAAAAMAAAIAAAMABAAAAAAAAwACAAABAAQCBAADAAACAgAEAwAC"
    "AAMEAAAAAwEEAQMAAAIAAgAAAAAAAAAEAAQAAwAABAECAAIAAAAAAgADAAACAwQABAAAAgMAAgAE"
    "AwAAAgACAAECAAEAAAECAQEBAAAABAACBAAAAAAAAAEAAAAEAQAAAAIAAgADAAEAAAAAAQAAAQME"
    "AgAAAAEEAAAAAAMAAQAAAAAEAAAEAQAAAAAAAAAAAAAAAAADAQQAAAAAAgABAAAAAAADAAADAAQA"
    "AAAAAwMAAAAEAAAAAAAAAAEAAAMAAAAAAAQAAAACAgAEAQAAAAABAAADAgABAAIAAAAAAwQCAAAD"
    "AgAAAAADAgAAAQAABAAABAAAAAAAAAIAAAEABAADAAAAAAAEAAAAAQEBAAAAAAMAAAIAAAAAAAMA"
    "AwIDAAEAAQQAAAIAAAEEAAECAAIAAAEAAAADAAIAAQICAAABAgAAAQAAAAIAAAADAAEDBAAAAQEA"
    "AgAAAAAEBAAAAAEAAgECAAIEAAAABAAEAQIABAAAAAAAAAAAAAMBAQAAAAMCAgADAAIDAwQDBAAE"
    "AAAAAAAAAAEAAAEAAwMAAAAAAAAAAAABAAAAAAAAAAEAAAADAgMAAAMAAAAAAAMAAQAAAAAAAgAA"
    "BAAAAAMBAQABAAAAAAAAAAIAAwAAAgAEAwABAAAAAAAAAAAAAAIAAgABAgAEAAABAQIAAgIDAgAE"
    "AAAAAAAAAQAABAAEAAAAAAAAAQIAAgAAAAMAAQACAAAAAAADAAQAAQABBAAEAAMABAABAQADAQAA"
    "AgABAgAEAAIAAAAAAgAAAwAAAwAAAAAEAAAAAAEAAAAAAAIEAAAAAgAABAEAAgAAAAAAAAEAAAAC"
    "AAECBAADAAAAAQAAAAIAAAAAAgMAAAAAAQAAAAQAAAAAAAMEAwEAAgEAAAAAAAAABAADAQIDAAAA"
    "AAEAAwAAAgAAAAEAAgAAAAAAAgAAAAAABAAEAAACAAIAAAQAAgADAAEAAAQAAAACAAECAwIEAAAA"
    "BAQAAAQABAMAAAQAAwIAAQMAAAQAAAACAAAEAAAABAAAAAAAAAMBAAEAAAQDAAAAAAQDAAAAAAIA"
    "AAAEAwACAAQAAgACAAACAQQAAAQDAgQDAQAAAAAEAAADBAECBAAEAAEBAAAAAAEAAgAAAwAAAgAB"
    "AwAAAgAEBAAAAAIEAAAAAwACAAIBAAABAwQAAQAAAAQAAAAAAAIAAAEBAAIAAAAAAAEAAAAAAAEB"
    "AAAAAgACAAAAAAMAAwAAAAAABAMABAMAAQQBAAQCAAEDAAAAAAIAAAAEAAMDAAAEAAEAAQAAAAAA"
    "AAICBAABAQQEAAAAAAQAAQABAAEEAAACBAAAAAMAAAAABAAAAAEBAAICAAIAAAAAAAAEBAAAAAMC"
    "AAQDAAABAAQCAAEAAAAABAQEAAIBAAAAAgAEAAEAAAIEBAACAAIAAAAABAMDBAQAAAAAAAIAAgAA"
    "AAACAAABAwMDAAAAAAAAAAACAQAAAwAAAAAEAAAAAAMAAAAAAgMAAAICAAMAAAAEAAAAAAABAAAA"
    "AAABAAAAAAMAAAEEAAIDAAEBAAQAAAMCAAAAAAAEAAACAAMAAAACAwAAAwAEAAAAAAQAAwABAAAC"
    "AwAAAAEABAQBAAIAAAIAAwAEAAEAAAACAgAAAAEEAAQAAAADAAMDAAQDBAABBAACAwAAAAAEAAMA"
    "AgQABAIAAAAEAAQCAQMAAAIBAAIAAAQEAAACAAEAAAAAAAEAAAABAAEAAAAABAAAAAAABAADAAAA"
    "BAABBAABAAADAAAAAAAAAAAAAQAAAAAAAAMAAQAAAQACAAAAAAACAAMAAAMAAwIBAAAABAAAAAMA"
    "AAAAAAABAAABAQIBAAAAAgAAAAAEAAAAAAQAAAAAAwAAAAAAAgAAAAAAAAAAAAACAgAAAAABBAAA"
    "AwACAAEDAAAAAAQAAQACAAAEAAAAAgAAAAIAAAMBAAAAAAIEAwAAAAQAAAMAAAMAAAAAAAAAAAMC"
    "BAQAAAMAAAEBAQAAAAAAAAIAAAMAAAMAAAAAAAIABAAAAAABAgAAAAAEAAQCAAIAAAIDAAMBAAAA"
    "AwAAAQADAwABAAADAAAEAwAAAAAABAMAAAEAAAAAAAAAAAAAAAAAAAAAAAACAAAAAAICAgACAAMA"
    "AAACAwAAAAIAAQAAAAAEAQAAAgAEAAEAAwAEAAAAAAAAAAQAAwAAAwAAAAQEAgAAAAMEAAAAAAAB"
    "AwQAAgADAgEDAAQDAAAAAAIAAAAAAAAAAAAABAQAAAEEBAABAAAAAQQAAAAABAAAAAMCAAAAAAAD"
    "BAAAAAEEAwIAAAADAAAAAAAEAAIAAAMBAAADAAAAAAAAAgAAAAMCAAAEAgACAAADAAAAAwABBAAD"
    "AAIAAAAAAQAABAADAAAAAAQAAQABAAMAAwADAAAAAAAAAAMEAwADAwQBAAAAAAMAAAAAAAEDAAAE"
    "AQAAAAAAAgAAAQAAAAICAAIEAAABBAACAAABAgAAAQAABAIDAgAEAAMAAAAAAAEEAAMDBAADBAAA"
    "BAAAAAADAAABAwADAAAAAAMAAAQAAQIAAAAAAwICAAIAAAIAAAAAAQAAAAICAAMAAAEAAgQAAAAA"
    "AAQAAAAABAAAAAEAAAIAAAAAAAAAAAAAAAMABAAAAAADAgAAAAAABAAABAAAAwICAAIAAAACBAAD"
    "AAAAAAADAAABAAAAAQAAAAACAgAEAAAAAAAEBAAAAAAAAAIABAQBAAAAAAAEAQAAAAIAAQADAAAD"
    "BAADAAAEBAQAAAACAAAEAAAEAAAEAAIBAAAAAgECAAAAAAMCAAIEAgADAAMAAAADAAEAAQAAAAAB"
    "BAADAQAAAAAAAQADAAAEBAIAAAIAAQIDAAACAwAAAAMAAAAAAAAAAAQABAMAAAIDAAABAgEAAAAB"
    "AAEBAAIEAwAABAACAAQAAwEAAAAAAAAAAAABAQAAAAMBBAMAAwQABAMABAAAAwMDAQQEAAABAAEB"
    "BAAAAAAAAAABAAEDAQQAAAAABAICAAIEAAMAAAAAAwADAAQDAAECAQAAAAAAAAAAAAMCAgAAAAIA"
    "AAQEAAAAAAEAAAAAAgEAAQQAAAAEBAQDBAICAAADAgIAAQAAAQABAgQCAAABAwAAAwABAAQDAAAA"
    "AAAEAAAAAgABAAAABAAABAAAAAAAAwAEAAAAAAMAAwAAAAAAAAABAAAAAwMAAQMAAAAAAgABAAAA"
    "AAMAAQAAAQACBAAAAQAAAAECAgMAAAAAAAMAAAAEAgAAAwQCAAIAAAIAAAAAAAADBAAAAQAAAAAA"
    "AAEEAAAAAAAAAgQAAAADAAADAAAAAAAAAAAAAAIBAAEEBAAAAAAEAAAAAwABAAIBAwAAAAMEAAAA"
    "AgIDBAMAAAABAAEAAAMBAAMCAAAAAAADAAIBAAADAAAAAAABAQAAAAIAAAAEAAEAAAAAAAAABAAE"
    "AAAAAAMAAgEAAQMAAAAAAAACAAMBAgABAwAAAAAEBAAAAQADAAEAAAMBAAAAAQIAAwABAgECAQMA"
    "AAAAAAACAAAAAAEAAAAAAAAEAAAAAAMEAwABAAAEAAAAAAAAAAECAQEAAAAAAAAAAAACAAAAAQAE"
    "AAQAAAACAAQAAAAAAAAAAAEAAAABAAQBAwIAAAAAAAQCAAEBAAIAAgAAAAMEAAAEAAACAQEAAAAA"
    "AAAAAAQAAQQCAAQEAgMDAAQAAAMAAAADAAAEAAEAAwAEBAQDAAACAAEAAAAABAMDAAMAAAEAAAQA"
    "AgMAAwAABAABAAIDAAQAAAICAAIAAAAAAAIEAgAAAgAEAwIAAAABAAAEAQAAAwAAAAACBAECAQAA"
    "AwAAAwQAAwQDAAAAAAACAQQDAAAAAAAEAAAAAwMBAAAAAAQAAAAAAgIAAAADBAADBAAEAAQABAAA"
    "BAAAAwQBAAAAAAACAAACAAIAAAAEAAEABAAAAgAAAAAAAAAAAAEEAAAAAwAAAQIAAAMAAQACAwQE"
    "AQABAwAAAAAAAAAAAAMBAAAABAIAAAAAAAIEAAAAAgAAAwAEAwADAAACAAEDAwQEAwAAAAAAAAAD"
    "AwACAAIDBAAABAAEAAAAAAACAgACAgICAAAAAAAAAAADAAIDAAQBAAMAAgAAAgAAAAAAAAAAAQAE"
    "AwQAAQAAAAIBAgAAAAEAAAQAAAAAAAIAAAABAQAAAwABBAADAwABAAIAAAAAAQQBAgIABAAAAAQC"
    "AAACAgMCAwQDAAAAAAACAAABAAICAAAAAgIAAAAAAQIAAAAAAAABAAAAAAAAAAAAAAIBBAQEAAQA"
    "AgQBAAEAAAAAAAAEAwAAAAAABAAAAQABAAAAAgAAAAEAAAMBAgMAAQAAAQAAAAQAAAQAAAAAAAAA"
    "AAEAAgIAAAIAAAAAAAAEAgAAAAIBAAAAAAAAAAIEAAAAAgIAAAQAAAAAAwAAAgIAAAIABAMAAQAA"
    "AAAAAAADAAAAAAAAAAADAQADBAAAAwAAAAAAAAABBAACAQAAAAABAgADAAAAAAAAAgADAAMAAAID"
    "AAIAAAAEAAAABAAAAAAAAwABAQECAwAAAAEAAAAAAAQAAAAAAAEEAAMAAAAEAAAAAAIAAwECAAAA"
    "AQAAAAABAAAAAAAABAAAAAQABAECAAIBAAECAAAAAAADAAACAgAEAAQAAAAAAAMABAAAAQEABAAA"
    "BAEAAwMEAAMAAAQABAQDBAAAAAAAAwAAAgEEAAABAAAAAAAAAAIDAgAEAQABAwACAAAEAQQEAAIA"
    "AAADAAABAgMEBAAAAAAAAgACAAAABAQAAAABAAAAAAMDAwEAAAAEAAMABAAEAwIAAAQAAQAEAAAA"
    "AgAAAAAAAAEAAAAAAAAAAwEAAAEAAgACAAAAAQADAAAAAAEAAAAAAAAABAECAAAAAAIAAAQBAgIA"
    "AwAAAAIAAAMAAAAEAAIAAAIAAQACAAAAAAAAAAAAAAMCAAADAAEBAgAAAwAAAwADAwADAAQAAAAA"
    "AAIBAwAAAQAAAAEAAAABAAAAAAAEAAEAAAQAAgQDAgEEAgMCBAAAAQIAAgAAAgIAAAABAAQAAAAA"
    "AAAAAAEAAAAAAwQAAAAAAwAEAAAAAAADAAAAAAAEAAABBAAAAAAAAwQEAAAAAgQAAAAEAgAAAAAA"
    "AAEAAAECAAAABAIEAAAAAgAAAAECAgAAAAMDAgAAAAIBAAAEAAAAAAAAAAQAAAMAAAAAAwAAAQQA"
    "AAEDAQADAAMAAAAAAAAAAAEAAAIEAAICAQAAAAIAAAAAAAEBAAEAAAAAAAACAAMDAAEAAQAAAAAA"
    "AAADAAADAAAAAAEBAwMBAwEAAAIBAAQAAAAAAAADAAAAAAEAAAMAAAABAwMAAAAAAwAABAAAAAAA"
    "AwIAAAIDBAAEAAAAAwIAAgAAAAAAAAAAAAIAAAAAAwADAAMABAMAAgQAAwAAAwAAAAAEAgADAQAE"
    "AAQAAgAEAAAAAAADAAMAAAADAgACAQQAAAAEAAEABAAAAwEABAABAgAEBAABAwMEAAAAAQAEAgEE"
    "AAMBAAAAAAAAAAAEAAAAAAEAAAABAAAAAwAAAQIAAAMAAAAAAAAAAAAAAAACAAACBAACAAAAAAIA"
    "AAICAAEAAQAAAwMAAwEBAwAEAAMDAAQCAAIEAAABBAABBAEEAAECAQMEAAAAAAACAwADBAIBAwAB"
    "AAAAAwACAgMCAAMAAAAAAwMAAAQAAAQAAQAAAAAAAAMABAQAAwAAAAEAAgABAAAABAEAAAAAAAAC"
    "AQIAAAAAAAMAAwIAAQACAQMEAwQAAAAEAAMAAQAAAAADAQABAAQAAAABAQMBAAAEAQAAAAAAAAAE"
    "AAAAAAIEAAAEAAAAAAAEAwEAAAAAAAIAAgAAAwEAAAEAAgAAAAMAAAQEAwAAAAADAQABAwAAAAAB"
    "AwADBAAEAQAAAwAABAAABAAAAAAAAAABAAAAAAMCAAAAAgEAAAQDAQAAAAMDAAAEAAIABAAAAAAA"
    "AQMEAAAAAAAAAAAAAAEEBAAEAAQDAAAAAAAAAgAAAAMAAwAAAAEAAAAAAgAAAQAAAgAEAAADBAAA"
    "AwABAAAAAwADAAICAAIAAAICAgMEAgAAAAAAAQACAAQBBAAAAQEBAAAAAAIAAAAAAgACAAIAAAAA"
    "AQAABAIDAAAAAAAAAAAAAAAEAAAAAAABAQAAAAAEAAAAAwABAwAAAAIEAAAABAEAAgMCAwACAAAC"
    "AAADAAAAAwAAAAMAAwMAAgACAAAAAAEDBAQAAwIDAAAAAAQCAgADAAADAgAAAAAAAwAAAAMBAQEA"
    "AwEAAwABAAAAAAMCAAAAAAADAAAABAQDBAAABAEAAwAAAAQEAAAAAwAAAgIBBAACAAABAAQAAAAD"
    "AAQABAICAAAEAQMAAAACBAEAAAIAAAMEAAAABAADAAAAAAIAAAMAAQAAAAABAAIAAAACAwMDAAAA"
    "AgACAAIEAAAAAAEEAAEAAAMDAAQEBAEAAAAAAAAAAAEAAgAEAAQAAAAEAAMABAABAQMAAQADAAID"
    "AAAAAAMCAgEAAwQAAgIAAAAEAAEAAAAAAAAABAAAAAAAAAQAAAAEAAAABAAAAAAAAAAAAAAAAAAA"
    "AAAEAwMAAQMAAwQAAQABAwACAAMAAAAAAAADAQAEAgAAAgIBAAQBBAAAAAAAAAQAAQAEAgAEAAIC"
    "AAIEAAIAAgAAAAADAAAABAQAAAACBAEEAwIABAACAAAAAAMABAABAAAAAAMAAAQAAAABAAMAAAAA"
    "AgACAAMAAAAAAwAAAAIAAAAAAAAAAAMEAAQEAAIAAQAAAAQDBAAAAAQABAMAAQQAAQAAAAEEAAMD"
    "AQAABAADAAAAAAABAgAAAAAABAIAAAABAAAABAABAgECAwMAAAACAgEABAABAAAAAgEBAAAEBAAC"
    "AAAAAgEAAAMAAAACAAAAAgMAAAAAAAQBAAAAAAACAQMCAAABAAADAAADAwABAAIAAAADAAADAQAA"
    "AAAABAACAAAAAAIAAAAABAMDBAQAAAAAAAQBAAQAAAAAAAAAAQAAAAEEAAMABAEAAAAEAgAAAAMA"
    "AAAAAgMCAgIAAAAAAgAAAAAAAAMAAAAAAAEAAAAAAgMBAAMAAAAABAMEAAQAAAMAAwACBAAEAAAB"
    "AAAAAAACBAQABAAEAgQAAAAEAQMDAAMAAAIEAQAEBAADAQIABAEDAAAAAgQABAADAAAAAgACBAMB"
    "AAMDAAAAAAAAAAIDAAAAAAIABAADAAAAAQAAAAAAAAAEAQAAAgABAAMDBAIBAAAABAADAAMEAwQA"
    "AAQCAAEAAwMAAAQBAAACAAABAAEAAAQCBAMBAgAAAAAAAAAABAQCAwMABAAAAAAAAAAAAAAAAQME"
    "AAAAAQAABAACAAMCAwEBAAACAgAAAgEAAAADAAAEBAAAAAAAAAABAAABAwMAAAMCAwAEAwIAAAQA"
    "BAICAAEBAAIAAAACAgIBAAAAAgQCAgAAAQQAAAAAAAAAAAMEAAADAwQABAACBAQAAwQAAQEDAQAA"
    "BAAAAAAAAwAAAAACAAMAAgMEAwEAAAAAAAEDAAAAAAIBAAQAAAMAAAMABAAEAAEEAwMAAAABBAAE"
    "AAIEAwAAAAAAAAMAAgQAAAMAAAEAAQIAAAMDBAAABAAAAAMAAAAEAAAEAAMAAAAAAAAAAAMAAAAE"
    "AAABAwAAAQAAAAEEAAAAAAIAAQAEAAAAAAADAAMAAAQDAAAAAgQCAgEAAAIBAAAAAAADBAIAAAMA"
    "AAQAAQQAAAACAAAAAAMAAgAAAQMAAAAAAQADAAIAAAAAAgAABAAAAAQEBAAEAQQAAwABAAACAAAA"
    "AAAAAAAAAAADAAAEAAABAgADAAIAAgEDAAADAAAAAAADAwQAAAMBAAAAAAAAAAAAAgABAQADAQQA"
    "BAAAAwAAAAABAAAAAAIDAAAAAwAEAAAAAQAAAAAAAwAAAAIDAAAAAwADAAQAAAEAAAECAAIABAAA"
    "BAAABAACAAMAAQAAAAIAAgIAAgAAAAQAAQACAAACAAABAAEBAAIDAAIABAAAAwEAAgMAAAAAAAMA"
    "BAACBAAAAAAABAABBAAEAAAAAQQAAQAAAAAEAgAAAAAAAwADAAAAAAAAAAMAAAAAAAEAAAAABAEA"
    "AAAEAgIAAAIAAAAAAAAAAAAAAAEEAAADAAAAAAEAAwAAAAMEAgAAAAAAAAAAAAIEAAEAAQAABAAA"
    "BAEAAAQAAwAAAwABAAIDAwQEAAAAAwQAAAQABAMAAAECAgACAAIDAAAAAQIEAAQABAQDAAAAAAAA"
    "AAAAAAAAAwABAwAAAQADAwIAAAAAAQABAAAAAAEABAQBAwABAAADAgAEAAIAAAMABAEAAAEAAQAA"
    "BAMAAwQCAwMAAQMCAwQAAwAAAAEABAAAAAEAAgEAAAAAAAAAAAAAAAAAAgAEAQAAAAEAAAAEAwAA"
    "AQIABAMEAAABAAMAAgEEAAIAAAEEAAABAAABAQAAAAAAAgIAAAAAAAADAgABBAMEAgACBAACBAQA"
    "AgADAAACAgQAAwADAwAEBAQAAAEBAAAABAECAAAAAAAABAACAAAEBAAAAAADAAAEAAMAAAIBAAAA"
    "AAQAAQAABAAAAAACAAEDAwAEBAAAAAAAAAACAQAAAAAEAAIAAAADAAAAAAIAAwAAAAEEBAAAAgAD"
    "AAAAAgEAAAQAAAEAAAAAAAIEAAMAAwQABAACAAEBAAEAAAEABAAAAAICBAQAAQAAAgIEAAAAAAAA"
    "AAAAAAAABAIBAAAAAgIAAAACAQAAAAABAAAAAAQEAgAEAAABAAAAAAAAAAEAAAMCAwAEBAMDBAAA"
    "AAABAAABAAEBAAABAwAAAAABAAABAwMAAAABAAMEAAAAAgAAAAQAAAACAAMAAAAAAAAAAAQAAAQD"
    "AAAABAABAAIAAAIAAAAAAAICAwACAwABAAAAAAQAAwADAgAAAAAAAgEABAIAAAAAAAABBAAAAAIC"
    "AAQAAAQAAAEAAwMDAAAAAQAEBAAAAAEAAAEBAAAAAgAAAwIABAADAAAEAgAAAAAABAAAAAAAAAAC"
    "AAQAAgAEAwAAAAAEAAMEBAEAAQACAAAEAAAABAAAAAAAAAAEAQQAAAQEAAQAAgAAAQEAAQAAAAQE"
    "AAABAAAAAAQABAAEAQAABAACAwACBAQEAAAAAQEAAQABAAAAAAAAAAAAAQAAAQAAAAAEAAACAAAA"
    "BAACAAEAAAAAAAMAAAIAAAMEAQAAAAIBAAIBAAAABAECAAAAAAAAAAABAAMBAAAAAwQAAgAAAwAA"
    "AwAEAQQAAwAAAQQAAwQAAAABAAABAAAEAAQAAAACAAABAAAAAAAAAQIAAAABAAAAAAICAAACAAIA"
    "AAADAgMCAAABAAAAAwACAAMABAAAAAAAAAAAAAAAAAIAAAAAAAQBAAAAAAECAQMBAAAAAAACAAAD"
    "AAAAAAQCAAQBAAACAAAAAAMAAwIAAgMAAAABAwMDBAAABAAEAAAAAAEBAAQCAQAEAAQABAIAAAID"
    "AAEAAQAAAAACAAQAAAABAAADAQECAAAAAAQAAAMABAACAAAAAAQAAAAAAAAAAQEDAAABAwQDAwIA"
    "BAAAAQADAAAAAgAEAwAABAABAQAABAABAAQAAgAAAAAAAAQAAAMBAAACBAAEAAEEAAAABAAABAAA"
    "AAAABAMDAAEBAAAAAAAEAgMAAAAEAgADAAACAgAAAAMAAAQBAQAAAQAEAgAAAAMDAAAAAAABBAAA"
    "AAAAAwQBAAIAAAABAAIAAAIABAMAAAAEAwMAAAABAAAAAwECBAAABAAAAAACAAAAAAAAAAAEAQIB"
    "AAAABAMAAAQCAwEBAgAAAAQAAQAAAAABAAAAAAIAAwACAwECAQAAAgMCAwAEAAAEAQQAAAAAAwAA"
    "AAMAAAMAAAAABAAAAAAAAAMAAAMEAAAAAAAEAAAAAAAAAAQAAwECAAQAAAAAAgAAAAAAAAAAAAAA"
    "AAAEAAADAwAAAAMCAAIAAAAAAwAAAgADAAACAAADAAAAAAMBAAEBAAECAAADAAAEAQMDBAACAAAC"
    "AAABAAACAAQAAAAAAQAAAAAAAQABAwQAAAQCAAAAAwMAAQADAAMAAAMAAAIAAAAAAAAAAAEEAAAA"
    "AAMAAAMEAAACAAAAAAMAAwIAAQMAAgIAAAIAAQAAAAAABAMAAAAAAgEAAAABAQEBAAQAAgQDAAAA"
    "BAMAAAEAAAAAAgIAAwMAAAAABAIAAAADAAECAgIAAAEBAAMBAAQAAgAAAAIAAAIAAAAAAAQEAAAD"
    "AQEEAQIDAAACAAACAAIEAAECAAAAAgMCAwACAAABAwAAAwAAAAAABAAEAAQDAAAAAAABAQEBAAAE"
    "AAAAAwAAAgAAAAADAAECAQMAAAABAAACAAAAAAAAAwMAAAIAAAIAAAEBAAIEAAAEAAAAAAAAAAMA"
    "AQQAAAMEAAMAAwMAAQAAAAAAAAMEAAQCAAIDAAMDBAQAAAAEAAEAAAMCAQACAgAAAAEDAAQAAwAA"
    "AAAAAQQAAAICBAMAAAEAAAAAAAQDAAAAAQAAAQADAAADAAAAAAAAAQAABAAAAAAAAQADAgICAQIA"
    "AAIBAAEAAwAAAAAAAAADAwAAAAAABAIAAAAAAAAEAAMABAAAAAAAAAQAAwQABAAAAAAAAAAAAwED"
    "AAMAAAAAAAAABAMAAAAAAwEAAgABAAAAAQAAAAACAAAAAAAEAQABAAABAQAAAQAAAAMAAgABAAMA"
    "AAAABAAEAQAAAAMABAAAAAEAAQAAAwQDAAACAAQEAAACAAAEBAAAAAMBAAABAAACAAAAAAQAAAAB"
    "AAADAQIBAAADAAEAAQAAAgMBAAADAAIDAAQAAAAAAQEBAQAAAgMAAAACAAAEAwABAAAAAAAEAAAD"
    "AAEEAwEAAQAAAQACAAEAAAMAAQMAAgAAAAIAAAQAAAAAAAIDAAAAAAA="
)


def _rank_to_b64():
    """(debug helper) regenerate _RANK_B64 with jax on CPU."""
    import jax
    import jax.numpy as jnp

    cpu = jax.devices("cpu")[0]
    with jax.default_device(cpu):
        k1, k2 = jax.random.split(jax.random.key(1))
        coin = jax.random.uniform(k1, (B,)) < 0.5
        rank = jnp.where(coin, 0, jax.random.randint(k2, (B,), 0, 5))
    return base64.b64encode(np.asarray(rank, dtype=np.uint8).tobytes()).decode()


_RANK_CACHE = None


def _get_rank() -> np.ndarray:
    """rank[i]: which of the 5 nearest negatives the reference picks per row.

    Must reproduce the reference's jax.random draws bit-exactly. The default
    PRNG impl here is "rbg", whose output is backend-dependent, so compute on
    the CPU backend (the grading reference runs on CPU). Falls back to the
    embedded constant (generated the same way) if jax is unavailable.
    """
    global _RANK_CACHE
    if _RANK_CACHE is not None:
        return _RANK_CACHE
    try:
        import jax
        import jax.numpy as jnp

        cpu = jax.devices("cpu")[0]
        with jax.default_device(cpu):
            k1, k2 = jax.random.split(jax.random.key(1))
            coin = jax.random.uniform(k1, (B,)) < 0.5
            rank = jnp.where(coin, 0, jax.random.randint(k2, (B,), 0, 5))
            r = np.asarray(jax.device_get(rank)).astype(np.uint8)
    except Exception:
        r = np.frombuffer(base64.b64decode(_RANK_B64), dtype=np.uint8)
    assert r.shape == (B,)
    _RANK_CACHE = r
    return r


_NC_CACHE = None


def _build_nc():
    import concourse.mybir as mybir
    import concourse.tile as tile
    from concourse import bacc
    from concourse.masks import make_identity

    F32 = mybir.dt.float32
    F16 = mybir.dt.float16
    F8 = mybir.dt.float8e4
    AF = mybir.ActivationFunctionType

    nc = bacc.Bacc()
    xs = nc.dram_tensor("xs", [M, 2 * D], F8, kind="ExternalInput").ap()
    oh = nc.dram_tensor("oh", [M, 8], F32, kind="ExternalInput").ap()
    dsel = nc.dram_tensor("dsel", [128, NG], F32, kind="ExternalInput").ap()
    loss = nc.dram_tensor("loss", [128, RB], F32, kind="ExternalOutput").ap()

    with tile.TileContext(nc) as tc:
        with (
            tc.tile_pool(name="const", bufs=1) as constp,
            tc.tile_pool(name="big", bufs=1) as bigp,
            tc.tile_pool(name="small", bufs=4) as smallp,
            tc.tile_pool(name="pst", bufs=2, space="PSUM") as pst,
            tc.tile_pool(name="psg", bufs=3, space="PSUM") as psg,
            tc.tile_pool(name="dram", bufs=1, space="DRAM") as dramp,
        ):
            # ---------------- constants ----------------
            identf = constp.tile([128, 128], F32)
            make_identity(nc, identf)
            ident16 = constp.tile([128, 128], F16)
            nc.scalar.copy(ident16, identf)
            dsel_sb = constp.tile([128, NG], F32)
            nc.scalar.dma_start(dsel_sb, dsel)
            oh_sb = constp.tile([128, RB * 8], F32)
            nc.scalar.dma_start(
                oh_sb.rearrange("p (r k) -> p r k", r=RB),
                oh.rearrange("(r p) k -> p r k", p=128),
            )
            # negsel[:, g*128:(g+1)*128] = I * dsel[g]  (NEG_BIG iff g == my core)
            negf = constp.tile([128, NG * 128], F32)
            negsel = constp.tile([128, NG * 128], F16)
            for g in range(NG):
                nc.gpsimd.tensor_scalar_mul(
                    negf[:, g * 128 : (g + 1) * 128], identf, dsel_sb[:, g : g + 1]
                )
            nc.scalar.copy(negsel, negf)

            # ---------------- slab load ----------------
            # fp8 over the wire; upcast once to fp16, then the rest of the
            # pipeline is identical to the fp16 version.
            # xs_sb[:, r*512 + 0:256]   = anchor rows block r (fp16)
            # xs_sb[:, r*512 + 256:512] = positive rows block r (fp16)
            xs8 = bigp.tile([128, RB * 2 * D], F8)
            nc.sync.dma_start(
                xs8.rearrange("p (r d) -> p r d", r=RB),
                xs.rearrange("(r p) d -> p r d", p=128),
            )
            xs_sb = bigp.tile([128, RB * 2 * D], F16)
            nc.vector.tensor_copy(xs_sb, xs8)

            # ---------------- local positives: norm + scale ----------------
            np2 = constp.tile([128, RB], F32)
            nps = constp.tile([128, RB], F32)
            invnp = constp.tile([128, RB], F32)
            sq = smallp.tile([128, D], F32, tag="sq")
            for r in range(RB):
                nc.scalar.activation(
                    sq,
                    xs_sb[:, r * 512 + 256 : r * 512 + 512],
                    AF.Square,
                    accum_out=np2[:, r : r + 1],
                )
            nc.scalar.activation(nps, np2, AF.Sqrt)
            nc.vector.reciprocal(invnp, nps)
            ps16 = bigp.tile([128, RB * D], F16)  # unit positives, fp16
            for r in range(RB):
                nc.gpsimd.tensor_scalar_mul(
                    ps16[:, r * D : (r + 1) * D],
                    xs_sb[:, r * 512 + 256 : r * 512 + 512],
                    invnp[:, r : r + 1],
                )

            # ---------------- local transposes (K-major operands) -----------
            # psl: [k-chunk, col] layout of local scaled positives ([128, 2*M])
            psl = bigp.tile([128, 2 * M], F16)
            for k in range(2):
                for r4 in range(2):
                    pt = pst.tile([128, 512], F16)
                    for j in range(4):
                        r = r4 * 4 + j
                        nc.tensor.transpose(
                            pt[:, j * 128 : (j + 1) * 128],
                            ps16[:, r * D + k * 128 : r * D + k * 128 + 128],
                            ident16,
                        )
                    nc.scalar.copy(
                        psl[:, k * M + r4 * 512 : k * M + (r4 + 1) * 512], pt
                    )
            aT = [bigp.tile([128, M], F16, name=f"aT{k}") for k in range(2)]
            for k in range(2):
                for r4 in range(2):
                    pt = pst.tile([128, 512], F16)
                    for j in range(4):
                        r = r4 * 4 + j
                        nc.tensor.transpose(
                            pt[:, j * 128 : (j + 1) * 128],
                            xs_sb[:, r * 512 + k * 128 : r * 512 + k * 128 + 128],
                            ident16,
                        )
                    nc.scalar.copy(aT[k][:, r4 * 512 : (r4 + 1) * 512], pt)

            # ---------------- s_ii and anchor norms (pre-CC, off-path) ------
            # s_ii = <a_raw16, p_unit16>: same operand bits as the matmul path.
            sii = constp.tile([128, RB], F32)
            for r in range(RB):
                dot = smallp.tile([128, D], F32, tag="dot")
                nc.gpsimd.tensor_mul(
                    dot,
                    xs_sb[:, r * 512 : r * 512 + 256],
                    ps16[:, r * D : (r + 1) * D],
                )
                nc.vector.reduce_sum(sii[:, r : r + 1], dot, axis=mybir.AxisListType.X)
            na2 = constp.tile([128, RB], F32)
            na_half = constp.tile([128, RB], F32)
            inv2na = constp.tile([128, RB], F32)
            sqa = smallp.tile([128, D], F32, tag="sqa")
            for r in range(RB):
                nc.scalar.activation(
                    sqa,
                    xs_sb[:, r * 512 : r * 512 + 256],
                    AF.Square,
                    accum_out=na2[:, r : r + 1],
                )
            nc.scalar.activation(na_half, na2, AF.Sqrt, scale=0.25)
            nc.vector.reciprocal(inv2na, na_half)

            # ---------------- AllGather the K-major positives ----------------
            pslab_d = dramp.tile([2, 128, M], F16)
            nc.sync.dma_start(
                pslab_d.rearrange("k p m -> p k m"),
                psl.rearrange("p (k m) -> p k m", k=2),
            )
            pall_d = nc.dram_tensor(
                "pall_d", [NG, 2, 128, M], F16, addr_space="Shared"
            ).ap()
            nc.gpsimd.collective_compute(
                "AllGather",
                mybir.AluOpType.bypass,
                replica_groups=[list(range(NCORES))],
                ins=[pslab_d.opt()],
                outs=[pall_d.opt()],
            )
            pT = [bigp.tile([128, B], F16, name=f"pT{k}") for k in range(2)]
            for g in range(NG):
                for k in range(2):
                    nc.sync.dma_start(pT[k][:, g * M : (g + 1) * M], pall_d[g, k])

            # ---------------- main loop: matmul granules + top-8 ------------
            cand = [
                constp.tile([128, NG * 8], F32, name=f"cand{r}") for r in range(RB)
            ]
            for g in range(NG):
                for r in range(RB):
                    gt = psg.tile([128, GW], F32)
                    hd = r // 4  # 512-col half holding this row block's diagonal
                    for h in range(2):
                        for k in range(2):
                            nc.tensor.matmul(
                                gt[:, h * 512 : (h + 1) * 512],
                                aT[k][:, r * 128 : (r + 1) * 128],
                                pT[k][:, g * GW + h * 512 : g * GW + (h + 1) * 512],
                                start=(k == 0),
                                stop=(k == 1 and h != hd),
                            )
                        if h == hd:
                            # accumulate dsel[g] * I at the self-match block:
                            # NEG_BIG on the diagonal iff granule g is mine.
                            nc.tensor.matmul(
                                gt[:, r * 128 : (r + 1) * 128],
                                negsel[:, g * 128 : (g + 1) * 128],
                                ident16,
                                start=False,
                                stop=True,
                            )
                    nc.vector.max(out=cand[r][:, g * 8 : (g + 1) * 8], in_=gt)

            # ---------------- epilogue: merge, select, loss -----------------
            top8a = constp.tile([128, RB * 8], F32)
            for r in range(RB):
                nc.vector.max(out=top8a[:, r * 8 : (r + 1) * 8], in_=cand[r])
            sel_all = constp.tile([128, RB * 8], F32)
            nc.vector.tensor_mul(sel_all, top8a, oh_sb)
            selv = constp.tile([128, RB], F32)
            nc.vector.reduce_sum(
                selv,
                sel_all.rearrange("p (r k) -> p r k", r=RB),
                axis=mybir.AxisListType.X,
            )
            loss_sb = constp.tile([128, RB], F32)
            nc.vector.tensor_sub(loss_sb, selv, sii)
            # fold the anchor scale in before relu: relu(c*x) = c*relu(x)
            nc.vector.tensor_mul(loss_sb, loss_sb, inv2na)
            relu_sb = constp.tile([128, RB], F32)
            nc.scalar.activation(relu_sb, loss_sb, AF.Relu)
            nc.sync.dma_start(loss, relu_sb)

    nc.compile()
    return nc


def _get_nc():
    global _NC_CACHE
    if _NC_CACHE is None:
        _NC_CACHE = _build_nc()
    return _NC_CACHE


def _host_inputs():
    """Constant (input-independent) host arrays: one-hot rank + diag selector."""
    rank = _get_rank()
    onehot = np.zeros((B, 8), dtype=np.float32)
    onehot[np.arange(B), rank] = 1.0
    dsel = np.zeros((NCORES * 128, NG), dtype=np.float32)
    for c in range(NCORES):
        dsel[c * 128 : (c + 1) * 128, c] = NEG_BIG
    return onehot, dsel


_RT_CACHE = None


def _get_rt():
    """Cached 8-core jitted executable + device-resident constant inputs.

    Mirrors bass2jax.run_bass_via_pjrt's multi-core path, but builds the
    jitted shard_map once per process (run_bass_via_pjrt re-creates the
    closure -> re-jits + re-NEFF-compiles on every call) and keeps the
    constant operands on device.
    """
    global _RT_CACHE
    if _RT_CACHE is not None:
        return _RT_CACHE

    import jax
    from jax.experimental.shard_map import shard_map
    from jax.sharding import Mesh, NamedSharding, PartitionSpec

    import concourse.mybir as mybir
    from concourse import bass2jax

    bass2jax.install_neuronx_cc_hook()
    nc = _get_nc()
    assert nc.dbg_addr is None
    partition_name = nc.partition_id_tensor.name if nc.partition_id_tensor else None

    in_names, out_names, out_avals, zero_outs = [], [], [], []
    for alloc in nc.m.functions[0].allocations:
        if not isinstance(alloc, mybir.MemoryLocationSet):
            continue
        name = alloc.memorylocations[0].name
        if alloc.kind == "ExternalInput":
            if name != partition_name:
                in_names.append(name)
        elif alloc.kind == "ExternalOutput":
            shape = tuple(alloc.tensor_shape)
            dtype = mybir.dt.np(alloc.dtype)
            out_names.append(name)
            out_avals.append(jax.core.ShapedArray(shape, dtype))
            zero_outs.append(np.zeros(shape, dtype))
    assert in_names == ["xs", "oh", "dsel"] and out_names == ["loss"], (
        in_names,
        out_names,
    )
    n_params = len(in_names)
    all_names = list(in_names) + list(out_names)
    if partition_name is not None:
        all_names.append(partition_name)
    all_names = tuple(all_names)

    def _body(*args):
        operands = list(args)
        if partition_name is not None:
            operands.append(bass2jax.partition_id_tensor())
        outs = bass2jax._bass_exec_p.bind(
            *operands,
            out_avals=tuple(out_avals),
            in_names=all_names,
            out_names=tuple(out_names),
            lowering_input_output_aliases=(),
            sim_require_finite=True,
            sim_require_nnan=True,
            nc=nc,
        )
        return tuple(outs)

    devices = jax.devices()[:NCORES]
    assert len(devices) == NCORES, f"need {NCORES} cores, got {len(devices)}"
    mesh = Mesh(np.asarray(devices), ("core",))
    spec = PartitionSpec("core")
    n_outs = len(out_names)
    # No donation: the kernel DMA-writes every element of `loss`, so the
    # output-storage operand can be a persistent on-device zeros array
    # (saves a 32 KB upload + an extra sync per call).
    sharded = jax.jit(
        shard_map(
            _body,
            mesh=mesh,
            in_specs=(spec,) * (n_params + n_outs),
            out_specs=(spec,) * n_outs,
            check_rep=False,
        ),
        keep_unused=True,
    )

    onehot, dsel = _host_inputs()
    sh = NamedSharding(mesh, spec)
    oh_dev = jax.device_put(onehot, sh)
    dsel_dev = jax.device_put(dsel, sh)
    zero_shape = (NCORES * zero_outs[0].shape[0], *zero_outs[0].shape[1:])
    zeros_dev = jax.device_put(np.zeros(zero_shape, zero_outs[0].dtype), sh)
    in_dtype = mybir.dt.np(mybir.dt.float8e4)

    _RT_CACHE = (sharded, oh_dev, dsel_dev, zeros_dev, devices, sh, in_dtype)
    return _RT_CACHE


def _run_fast(x32: np.ndarray) -> np.ndarray:
    import jax

    sharded, oh_dev, dsel_dev, zeros_dev, devices, sh, in_dtype = _get_rt()
    # Pipeline the fp8 cast with the per-device uploads: cast slab c on the
    # host while slab c-1 is in flight (device_put is async).
    shards = []
    for c in range(NCORES):
        chunk = np.asarray(x32[c * M : (c + 1) * M], dtype=in_dtype)
        shards.append(jax.device_put(chunk, devices[c]))
    xs_dev = jax.make_array_from_single_device_arrays((B, 2 * D), sh, shards)
    out = sharded(xs_dev, oh_dev, dsel_dev, zeros_dev)
    return np.asarray(out[0])  # [NCORES*128, RB] per-row losses


def _run_spmd(x32: np.ndarray) -> np.ndarray:
    """Fallback: the stock run_bass_kernel_spmd path (re-jits per call)."""
    import concourse.mybir as mybir
    from concourse.bass_utils import run_bass_kernel_spmd

    in_dtype = mybir.dt.np(mybir.dt.float8e4)
    onehot, dsel = _host_inputs()
    in_maps = []
    for c in range(NCORES):
        in_maps.append(
            {
                "xs": np.asarray(x32[c * M : (c + 1) * M], dtype=in_dtype),
                "oh": np.ascontiguousarray(onehot[c * M : (c + 1) * M]),
                "dsel": np.ascontiguousarray(dsel[c * 128 : (c + 1) * 128]),
            }
        )
    res = run_bass_kernel_spmd(_get_nc(), in_maps, list(range(NCORES)))
    return np.concatenate([res.results[c]["loss"] for c in range(NCORES)], axis=0)


def kernel(x: np.ndarray, _want_timing: bool = False):
    """x: [8192, 2, 256] float32 -> scalar float32 loss (0-d ndarray)."""
    x = np.asarray(x)
    assert x.shape == (B, 2, D)
    x32 = np.ascontiguousarray(x.reshape(B, 2 * D))

    try:
        per_row = _run_fast(x32)
    except Exception as e:  # pragma: no cover - belt and braces
        import sys

        print(f"kernel: fast path failed ({type(e).__name__}: {e}); "
              f"falling back to run_bass_kernel_spmd", file=sys.stderr)
        per_row = _run_spmd(x32)
    # per_row[c*128 + p, r] = loss of global row c*1024 + r*128 + p; the mean
    # over all entries is permutation-invariant.
    out = np.float32(per_row.mean(dtype=np.float64))
    if _want_timing:
        return np.asarray(out), None, per_row
    return np.asarray(out)


if __name__ == "__main__":
    rng = np.random.default_rng(0)
    x = rng.standard_normal((B, 2, D)).astype(np.float32)
    print(kernel(x))
